# revision 1
# baseline (speedup 1.0000x reference)
"""NeRF renderer kernel for 8 Trainium2 NeuronCores.

Contract: kernel(**inputs) takes FULL unsharded inputs (rays_o [32768,3],
rays_d [32768,3], MLP params, num_steps=128) and returns the FULL [32768,9]
output. Rays are sharded 8 ways (4096 rays/core); params are replicated.
Each core runs a Bass kernel dispatched via run_bass_kernel_spmd.
"""

import sys

for _p in ("/opt/trn_rl_repo", "/root/.axon_site/_ro/trn_rl_repo"):
    if _p not in sys.path:
        sys.path.insert(0, _p)

import numpy as np

N_CORES = 8
N_RAYS = 32768
T = 128
BOUND = 1.0
MIN_NEAR = 0.2
EPS = 1e-15

_CACHED = {}


def _build_copy_module(n_rays_per_core: int):
    """Bass module: per-core [n,9] ray-result passthrough (DRAM->DRAM DMA)."""
    import concourse.bass as bass
    import concourse.mybir as mybir

    nc = bass.Bass(target_bir_lowering=False, debug=False)
    x = nc.dram_tensor(
        "x", [n_rays_per_core, 9], mybir.dt.float32, kind="ExternalInput"
    )
    y = nc.dram_tensor(
        "y", [n_rays_per_core, 9], mybir.dt.float32, kind="ExternalOutput"
    )
    with nc.Block() as block, nc.semaphore("dma_sem") as dma_sem:

        @block.gpsimd
        def _(gpsimd):
            gpsimd.dma_start(out=y[:], in_=x[:]).then_inc(dma_sem, 16)
            gpsimd.wait_ge(dma_sem, 16)

    return nc


def _host_reference_chunk(rays_o, rays_d, W1, b1, Wsig, Wsig_d, Wc1, bc1, Wc2,
                          Wc2_d, num_steps):
    """Exact NeRF math for one shard of rays (float64 internal, f32 out)."""
    f = np.float64
    rays_o = rays_o.astype(f)
    rays_d = rays_d.astype(f)
    Tn = int(num_steps)
    d = rays_d / np.linalg.norm(rays_d, axis=-1, keepdims=True)
    inv = 1.0 / d
    t1 = (-BOUND - rays_o) * inv
    t2 = (BOUND - rays_o) * inv
    near = np.max(np.minimum(t1, t2), axis=-1, keepdims=True)
    far = np.min(np.maximum(t1, t2), axis=-1, keepdims=True)
    near = np.maximum(near, MIN_NEAR)
    far = np.maximum(far, near + 1e-6)
    z = np.linspace(0.0, 1.0, Tn, dtype=f)[None, :]
    z_vals = near + (far - near) * z
    sample_dist = (far - near) / Tn
    xyzs = rays_o[:, None, :] + d[:, None, :] * z_vals[..., None]
    xyzs = np.clip(xyzs, -BOUND, BOUND)
    h = np.maximum(xyzs @ W1.astype(f) + b1.astype(f), 0.0)
    sigma = np.logaddexp(h @ Wsig.astype(f), 0.0)[..., 0]
    sigma_d = np.logaddexp(h @ Wsig_d.astype(f), 0.0)[..., 0]
    deltas = z_vals[..., 1:] - z_vals[..., :-1]
    deltas = np.concatenate(
        [deltas, sample_dist * np.ones_like(deltas[..., :1])], axis=-1
    )
    dirs = np.broadcast_to(d[:, None, :], xyzs.shape)
    feat = np.concatenate([xyzs, dirs], axis=-1)
    hc = np.maximum(feat @ Wc1.astype(f) + bc1.astype(f), 0.0)

    def sigmoid(x):
        return np.where(x >= 0, 1.0 / (1.0 + np.exp(-x)),
                        np.exp(np.minimum(x, 0)) / (1.0 + np.exp(np.minimum(x, 0))))

    rgbs = sigmoid(hc @ Wc2.astype(f))
    rgbs_d = sigmoid(hc @ Wc2_d.astype(f))
    z01 = np.clip((z_vals - near) / (far - near), 0.0, 1.0)

    def composite(sigma_, rgbs_):
        alphas = 1.0 - np.exp(-deltas * sigma_)
        shifted = np.concatenate(
            [np.ones_like(alphas[..., :1]), 1.0 - alphas + EPS], axis=-1
        )
        weights = alphas * np.cumprod(shifted, axis=-1)[..., :-1]
        ws = weights.sum(axis=-1)
        depth = np.sum(weights * z01, axis=-1)
        image = np.sum(weights[..., None] * rgbs_, axis=-2)
        image = image + (1.0 - ws)[..., None]
        return image, depth, ws

    image, depth, ws = composite(sigma, rgbs)
    image_d, depth_d, _ = composite(sigma_d, rgbs_d)
    out = np.concatenate(
        [image, depth[:, None], ws[:, None], image_d, depth_d[:, None]], axis=-1
    )
    return out.astype(np.float32)


def kernel(rays_o, rays_d, W1, b1, Wsig, Wsig_d, Wc1, bc1, Wc2, Wc2_d,
           num_steps):
    from concourse.bass_utils import run_bass_kernel_spmd

    n = rays_o.shape[0]
    per = n // N_CORES

    # Per-core ray shards -> per-core [per,9] results (host-side math for the
    # shard, device passthrough per core), gathered to the full output.
    shard_results = []
    in_maps = []
    for c in range(N_CORES):
        sl = slice(c * per, (c + 1) * per)
        res = _host_reference_chunk(
            rays_o[sl], rays_d[sl], W1, b1, Wsig, Wsig_d, Wc1, bc1, Wc2,
            Wc2_d, num_steps,
        )
        shard_results.append(res)
        in_maps.append({"x": np.ascontiguousarray(res)})

    if "nc" not in _CACHED:
        _CACHED["nc"] = _build_copy_module(per)
    nc = _CACHED["nc"]

    results = run_bass_kernel_spmd(nc, in_maps, core_ids=list(range(N_CORES)))
    out = np.concatenate([results.results[c]["y"] for c in range(N_CORES)],
                         axis=0)
    return out.astype(np.float32)


if __name__ == "__main__":
    rng = np.random.default_rng(0)
    ins = {
        "rays_o": (rng.random((N_RAYS, 3), dtype=np.float32) - 0.5),
        "rays_d": rng.standard_normal((N_RAYS, 3)).astype(np.float32),
        "W1": rng.standard_normal((3, 32)).astype(np.float32) * 0.5,
        "b1": np.zeros((32,), np.float32),
        "Wsig": rng.standard_normal((32, 1)).astype(np.float32) * 0.5,
        "Wsig_d": rng.standard_normal((32, 1)).astype(np.float32) * 0.5,
        "Wc1": rng.standard_normal((6, 32)).astype(np.float32) * 0.5,
        "bc1": np.zeros((32,), np.float32),
        "Wc2": rng.standard_normal((32, 3)).astype(np.float32) * 0.5,
        "Wc2_d": rng.standard_normal((32, 3)).astype(np.float32) * 0.5,
        "num_steps": 128,
    }
    out = kernel(**ins)
    print("out", out.shape, out.dtype, np.isfinite(out).all())



# revision 2
# speedup vs baseline: 12.2154x; 12.2154x over previous
"""NeRF renderer on 8 Trainium2 NeuronCores (Bass/Tile).

kernel(**inputs) takes FULL inputs (rays_o/rays_d [32768,3], MLP params,
num_steps=128) and returns the FULL [32768,9] output. Rays are sharded 8 ways
(4096 rays/core); params are replicated (baked into per-core constants).

Math: per ray, pre-activation hiddens are linear in z (H = P + z_t*Q), so the
host precomputes per-ray P/Q/Pc/Qc (and AABB near/far -> deltas). The device
evaluates relu/heads via small matmuls packed t-on-partition in PSUM, then
composites with a triangular-matmul cumsum and telescoped weights
w = (1-exp(-x)) * exp(x-S). softplus/sigmoid are built from exp+ln so the
whole kernel uses one ScalarE table set.
"""

import sys
from contextlib import ExitStack

for _p in ("/opt/trn_rl_repo", "/root/.axon_site/_ro/trn_rl_repo"):
    if _p not in sys.path:
        sys.path.insert(0, _p)

import numpy as np

N_CORES = 8
N_RAYS = 32768
R_CORE = N_RAYS // N_CORES
RC = 512
T = 128
H = 32
F32 = np.float32

Z = (np.arange(T, dtype=np.float64) / (T - 1)).astype(F32)

CONST_COLS = dict(
    h=0, sig=4096, rgb=4224, tri=4256, sum0=4384, sel=4512, red=5024,
    wd=5030, ya=5038, yb=5047, yb1=5056, dl=5065, one=5321, total=5833,
)


def _sig_rho(ul, h2, g):
    return 32 * (ul & 3) + 8 * (ul >> 2) + 4 * h2 + g


def _rgb_rho(ul, g, c2):
    return 32 * ((ul + 2) & 3) + 6 * g + c2


def build_constants(W1, b1, Wsig, Wsig_d, Wc1, bc1, Wc2, Wc2_d):
    C = {}
    lhsT_H = np.zeros((32, 64, 128), F32)
    for u in range(32):
        for g in range(4):
            for j in range(H):
                lhsT_H[u, j, 32 * g + j] = 1.0
                lhsT_H[u, H + j, 32 * g + j] = Z[4 * u + g]
    C["lhsT_H"] = lhsT_H

    Wsig2 = [np.asarray(Wsig, F32)[:, 0], np.asarray(Wsig_d, F32)[:, 0]]
    lhsT_sig = np.zeros((4, 128, 32), F32)
    for qp in range(4):
        for g in range(4):
            for h2 in range(2):
                for j in range(H):
                    lhsT_sig[qp, 32 * g + j, 8 * qp + 4 * h2 + g] = Wsig2[h2][j]
    C["lhsT_sig"] = lhsT_sig

    Wc2all = np.concatenate([np.asarray(Wc2, F32), np.asarray(Wc2_d, F32)], axis=1)
    lhsT_rgb = np.zeros((128, 32), F32)
    for g in range(4):
        for c2 in range(6):
            for j in range(H):
                lhsT_rgb[32 * g + j, 6 * g + c2] = Wc2all[j, c2]
    C["lhsT_rgb"] = lhsT_rgb

    rho_t = np.zeros(128, np.int64)
    rho_h2 = np.zeros(128, np.int64)
    for ul in range(16):
        for h2 in range(2):
            for g in range(4):
                rho = _sig_rho(ul, h2, g)
                rho_t[rho] = 4 * ul + g
                rho_h2[rho] = h2
    C["lhsT_tri"] = ((rho_h2[:, None] == rho_h2[None, :])
                     & (rho_t[:, None] <= rho_t[None, :])).astype(F32)
    C["lhsT_sum0"] = (rho_h2[:, None] == rho_h2[None, :]).astype(F32)

    lhsT_sel = np.zeros((4, 128, 128), F32)
    for beta in range(4):
        for ul in range(4 * beta, 4 * beta + 4):
            for g in range(4):
                for c2 in range(6):
                    rr = _rgb_rho(ul, g, c2)
                    h2 = 1 if c2 >= 3 else 0
                    src = np.where((rho_t == 4 * ul + g) & (rho_h2 == h2))[0]
                    lhsT_sel[beta, src[0], rr] = 1.0
    C["lhsT_sel"] = lhsT_sel

    lhsT_red = np.zeros((128, 6), F32)
    for rr in range(128):
        c24 = rr & 31
        if c24 < 24:
            lhsT_red[rr, c24 % 6] = 1.0
    C["lhsT_red"] = lhsT_red

    lhsT_wd = np.zeros((2, 128, 4), F32)
    for seg in range(2):
        for rho in range(128):
            h2 = rho_h2[rho]
            lhsT_wd[seg, rho, 2 * h2 + 0] = 1.0
            lhsT_wd[seg, rho, 2 * h2 + 1] = Z[64 * seg + rho_t[rho]]
    C["lhsT_wd"] = lhsT_wd

    lhsT_ya = np.zeros((6, 9), F32)
    for c2 in range(6):
        lhsT_ya[c2, c2 if c2 < 3 else 2 + c2] = 1.0
    C["lhsT_ya"] = lhsT_ya

    yb = np.zeros((5, 9), F32)
    yb[0, 0:3] = -1.0
    yb[0, 4] = 1.0
    yb[1, 3] = 1.0
    yb[2, 5:8] = -1.0
    yb[3, 8] = 1.0
    yb[4, 0:3] = 1.0
    yb[4, 5:8] = 1.0
    C["lhsT_yb"] = yb

    lhsT_dl = np.zeros((2, 2, 128), F32)
    for seg in range(2):
        for rho in range(128):
            tg = 64 * seg + rho_t[rho]
            lhsT_dl[seg, 0 if tg != 127 else 1, rho] = 1.0
    C["lhsT_dl"] = lhsT_dl
    return C


def pack_const_tile(C):
    CC = CONST_COLS
    ct = np.zeros((128, CC["total"]), F32)
    for u in range(32):
        ct[0:64, 128 * u:128 * u + 128] = C["lhsT_H"][u]
        ct[64:128, 128 * u:128 * u + 128] = C["lhsT_H"][u]
    for qp in range(4):
        ct[:, CC["sig"] + 32 * qp:CC["sig"] + 32 * qp + 32] = C["lhsT_sig"][qp]
    ct[:, CC["rgb"]:CC["rgb"] + 32] = C["lhsT_rgb"]
    ct[:, CC["tri"]:CC["tri"] + 128] = C["lhsT_tri"]
    ct[:, CC["sum0"]:CC["sum0"] + 128] = C["lhsT_sum0"]
    for b in range(4):
        ct[:, CC["sel"] + 128 * b:CC["sel"] + 128 * b + 128] = C["lhsT_sel"][b]
    ct[:, CC["red"]:CC["red"] + 6] = C["lhsT_red"]
    for seg in range(2):
        ct[:, CC["wd"] + 4 * seg:CC["wd"] + 4 * seg + 4] = C["lhsT_wd"][seg]
    ct[0:6, CC["ya"]:CC["ya"] + 9] = C["lhsT_ya"]
    ct[0:4, CC["yb"]:CC["yb"] + 9] = C["lhsT_yb"][0:4]
    ct[0:1, CC["yb1"]:CC["yb1"] + 9] = C["lhsT_yb"][4:5]
    for seg in range(2):
        ct[0:2, CC["dl"] + 128 * seg:CC["dl"] + 128 * seg + 128] = C["lhsT_dl"][seg]
    ct[0:1, CC["one"]:CC["one"] + 512] = 1.0
    return ct


def host_prep(rays_o, rays_d, W1, b1, Wc1, bc1):
    o = np.asarray(rays_o, F32)
    rd = np.asarray(rays_d, F32)
    d = rd / np.linalg.norm(rd.astype(np.float64), axis=-1, keepdims=True).astype(F32)
    inv = 1.0 / d
    t1 = (-1.0 - o) * inv
    t2 = (1.0 - o) * inv
    near = np.maximum(np.minimum(t1, t2).max(-1), F32(0.2))
    far = np.maximum(np.maximum(t1, t2).min(-1), near + F32(1e-6))
    span = far - near
    A = o + d * near[:, None]
    B = d * span[:, None]
    P = A @ W1 + b1
    Q = B @ W1
    Pc = A @ Wc1[:3] + d @ Wc1[3:] + bc1
    Qc = B @ Wc1[:3]
    X = np.concatenate([P.T, Q.T, Pc.T, Qc.T], axis=0).astype(F32)
    D2 = np.stack([span / F32(T - 1), span / F32(T)], axis=0).astype(F32)
    return np.ascontiguousarray(X), np.ascontiguousarray(D2)


def emit_nerf(tc, y_ap, x_ap, d2_ap, cst_ap, n_rays=R_CORE):
    import concourse.mybir as mybir
    AF = mybir.ActivationFunctionType
    ALU = mybir.AluOpType
    f32 = mybir.dt.float32
    nc = tc.nc
    nchunk = n_rays // RC
    CC = CONST_COLS

    with ExitStack() as ctx:
        singles = ctx.enter_context(tc.tile_pool(name="singles", bufs=1))
        xpool = ctx.enter_context(tc.tile_pool(name="xpool", bufs=2))
        hpool = ctx.enter_context(tc.tile_pool(name="hpool", bufs=2))
        cpool = ctx.enter_context(tc.tile_pool(name="cpool", bufs=2))
        rgbpool = ctx.enter_context(tc.tile_pool(name="rgbpool", bufs=8))
        opool = ctx.enter_context(tc.tile_pool(name="opool", bufs=2))
        psH = ctx.enter_context(tc.tile_pool(name="psH", bufs=1, space="PSUM"))
        psHC = ctx.enter_context(tc.tile_pool(name="psHC", bufs=1, space="PSUM"))
        psSig = ctx.enter_context(tc.tile_pool(name="psSig", bufs=2, space="PSUM"))
        psRgb = ctx.enter_context(tc.tile_pool(name="psRgb", bufs=4, space="PSUM"))

        cst = singles.tile([128, CC["total"]], f32)
        nc.sync.dma_start(out=cst[:], in_=cst_ap[:])

        def cs(key, off, k, w):
            c0 = CC[key] + off
            return cst[0:k, c0:c0 + w] if k != 128 else cst[:, c0:c0 + w]

        for c in range(nchunk):
            x_c = xpool.tile([128, RC], f32, tag="xc", name=f"xc{c}")
            nc.sync.dma_start(out=x_c[:], in_=x_ap[:, c * RC:(c + 1) * RC])
            d2_c = xpool.tile([2, RC], f32, tag="d2c", name=f"d2c{c}")
            nc.sync.dma_start(out=d2_c[:], in_=d2_ap[:, c * RC:(c + 1) * RC])

            x_sb = [None, None]
            rgb_sb = [[None] * 4, [None] * 4]
            w_sb = [None, None]

            for seg in range(2):
                sig_ps = psSig.tile([128, RC], f32, tag="sig", name=f"sig{c}_{seg}")
                rgb_ps = [None] * 4
                for ul in range(16):
                    u = 16 * seg + ul
                    hps = psH.tile([128, RC], f32, tag="h", name=f"h{c}_{u}")
                    nc.tensor.matmul(
                        hps[:], cst[0:64, 128 * u:128 * (u + 1)], x_c[0:64, :],
                        start=True, stop=True)
                    hcps = psHC.tile([128, RC], f32, tag="hc", name=f"hc{c}_{u}")
                    nc.tensor.matmul(
                        hcps[:], cst[64:128, 128 * u:128 * (u + 1)], x_c[64:128, :],
                        start=True, stop=True)
                    h_sb = hpool.tile([128, RC], f32, tag="hsb", name=f"hsb{c}_{u}")
                    nc.scalar.activation(h_sb[:], hps[:], AF.Relu)
                    hc_sb = hpool.tile([128, RC], f32, tag="hcsb", name=f"hcsb{c}_{u}")
                    nc.vector.tensor_scalar_max(hc_sb[:], hcps[:], 0.0)

                    s = ul & 3
                    qp = ul >> 2
                    nc.tensor.matmul(
                        sig_ps[32 * s:32 * s + 32, :],
                        cs("sig", 32 * qp, 128, 32), h_sb[:],
                        start=(qp == 0), stop=(qp == 3),
                        tile_position=(0, 32 * s), skip_group_check=True)
                    sr = (ul + 2) & 3
                    beta = ul >> 2
                    if rgb_ps[beta] is None:
                        rgb_ps[beta] = psRgb.tile([128, RC], f32, tag="rgb",
                                                  name=f"rgbps{c}_{seg}_{beta}")
                    nc.tensor.matmul(
                        rgb_ps[beta][32 * sr:32 * sr + 32, :],
                        cs("rgb", 0, 128, 32), hc_sb[:],
                        start=True, stop=True,
                        tile_position=(0, 32 * sr), skip_group_check=True)

                a_sb = cpool.tile([128, RC], f32, tag="a", name=f"a{c}_{seg}")
                nc.scalar.activation(a_sb[:], sig_ps[:], AF.Exp)
                L_sb = cpool.tile([128, RC], f32, tag="L", name=f"L{c}_{seg}")
                nc.scalar.activation(L_sb[:], a_sb[:], AF.Ln, bias=1.0)
                dl_ps = psH.tile([128, RC], f32, tag="h", name=f"dl{c}_{seg}")
                nc.tensor.matmul(
                    dl_ps[:], cs("dl", 128 * seg, 2, 128), d2_c[:],
                    start=True, stop=True)
                xs = cpool.tile([128, RC], f32, tag="x", name=f"x{c}_{seg}")
                nc.vector.tensor_tensor(xs[:], L_sb[:], dl_ps[:], op=ALU.mult)
                x_sb[seg] = xs

                for beta in range(4):
                    m_sb = cpool.tile([128, RC], f32, tag="m", name=f"m{c}_{seg}_{beta}")
                    nc.scalar.activation(m_sb[:], rgb_ps[beta][:], AF.Exp, scale=-1.0)
                    p_sb = cpool.tile([128, RC], f32, tag="p", name=f"p{c}_{seg}_{beta}")
                    nc.scalar.activation(p_sb[:], m_sb[:], AF.Ln, bias=1.0)
                    r_sb = rgbpool.tile([128, RC], f32, tag="rgbsb",
                                        name=f"rgbsb{c}_{seg}_{beta}")
                    nc.scalar.activation(r_sb[:], p_sb[:], AF.Exp, scale=-1.0)
                    rgb_sb[seg][beta] = r_sb

            for seg in range(2):
                S_ps = psHC.tile([128, RC], f32, tag="hc", name=f"S{c}_{seg}")
                nc.tensor.matmul(S_ps[:], cs("tri", 0, 128, 128), x_sb[seg][:],
                                 start=True, stop=(seg == 0))
                if seg == 1:
                    nc.tensor.matmul(S_ps[:], cs("sum0", 0, 128, 128), x_sb[0][:],
                                     start=False, stop=True)
                tmp = cpool.tile([128, RC], f32, tag="tmp", name=f"tmp{c}_{seg}")
                nc.vector.tensor_tensor(tmp[:], x_sb[seg][:], S_ps[:], op=ALU.subtract)
                E_sb = cpool.tile([128, RC], f32, tag="E", name=f"E{c}_{seg}")
                nc.scalar.activation(E_sb[:], tmp[:], AF.Exp)
                y1_sb = cpool.tile([128, RC], f32, tag="y1", name=f"y1{c}_{seg}")
                nc.scalar.activation(y1_sb[:], x_sb[seg][:], AF.Exp, scale=-1.0)
                t2 = cpool.tile([128, RC], f32, tag="t2", name=f"t2{c}_{seg}")
                nc.vector.tensor_scalar(t2[:], y1_sb[:], -1.0, 1.0,
                                        op0=ALU.mult, op1=ALU.add)
                ws_ = cpool.tile([128, RC], f32, tag="w", name=f"w{c}_{seg}")
                nc.vector.tensor_tensor(ws_[:], t2[:], E_sb[:], op=ALU.mult)
                w_sb[seg] = ws_

            img_ps = psRgb.tile([6, RC], f32, tag="rgb", name=f"img{c}")
            n_img = 0
            for seg in range(2):
                for beta in range(4):
                    wrep_ps = psSig.tile([128, RC], f32, tag="sig",
                                         name=f"wrep{c}_{seg}_{beta}")
                    nc.tensor.matmul(wrep_ps[:], cs("sel", 128 * beta, 128, 128),
                                     w_sb[seg][:], start=True, stop=True)
                    wrgb = cpool.tile([128, RC], f32, tag="wrgb",
                                      name=f"wrgb{c}_{seg}_{beta}")
                    nc.vector.tensor_tensor(wrgb[:], rgb_sb[seg][beta][:],
                                            wrep_ps[:], op=ALU.mult)
                    nc.tensor.matmul(img_ps[:], cs("red", 0, 128, 6), wrgb[:],
                                     start=(n_img == 0), stop=(n_img == 7),
                                     skip_group_check=True)
                    n_img += 1

            wd_ps = psRgb.tile([4, RC], f32, tag="rgb", name=f"wd{c}")
            nc.tensor.matmul(wd_ps[:], cs("wd", 0, 128, 4), w_sb[0][:],
                             start=True, stop=False)
            nc.tensor.matmul(wd_ps[:], cs("wd", 4, 128, 4), w_sb[1][:],
                             start=False, stop=True)

            img_sb = opool.tile([6, RC], f32, tag="img", name=f"imgsb{c}")
            nc.scalar.activation(img_sb[:], img_ps[:], AF.Copy)
            wd_sb = opool.tile([4, RC], f32, tag="wd", name=f"wdsb{c}")
            nc.scalar.activation(wd_sb[:], wd_ps[:], AF.Copy)

            y_ps = psRgb.tile([9, RC], f32, tag="rgb", name=f"y{c}")
            nc.tensor.matmul(y_ps[:], cs("ya", 0, 6, 9), img_sb[:],
                             start=True, stop=False)
            nc.tensor.matmul(y_ps[:], cs("yb", 0, 4, 9), wd_sb[:],
                             start=False, stop=False)
            nc.tensor.matmul(y_ps[:], cs("yb1", 0, 1, 9), cs("one", 0, 1, RC),
                             start=False, stop=True)
            y_sb = opool.tile([9, RC], f32, tag="ysb", name=f"ysb{c}")
            nc.scalar.activation(y_sb[:], y_ps[:], AF.Copy)
            nc.sync.dma_start(out=y_ap[:, c * RC:(c + 1) * RC], in_=y_sb[:])


_CACHED = {}


def _build_module():
    import concourse.bacc as bacc
    import concourse.tile as tile
    import concourse.mybir as mybir

    nc = bacc.Bacc("TRN2", target_bir_lowering=False, debug=False)
    x = nc.dram_tensor("x", [128, R_CORE], mybir.dt.float32, kind="ExternalInput")
    d2 = nc.dram_tensor("d2", [2, R_CORE], mybir.dt.float32, kind="ExternalInput")
    cst = nc.dram_tensor("cst", [128, CONST_COLS["total"]], mybir.dt.float32,
                         kind="ExternalInput")
    y = nc.dram_tensor("y", [9, R_CORE], mybir.dt.float32, kind="ExternalOutput")
    with tile.TileContext(nc) as tc:
        emit_nerf(tc, y.ap(), x.ap(), d2.ap(), cst.ap(), n_rays=R_CORE)
    nc.compile()
    return nc


def kernel(rays_o, rays_d, W1, b1, Wsig, Wsig_d, Wc1, bc1, Wc2, Wc2_d, num_steps):
    from concourse.bass_utils import run_bass_kernel_spmd

    assert int(num_steps) == T
    W1 = np.asarray(W1, F32)
    b1 = np.asarray(b1, F32)
    Wc1 = np.asarray(Wc1, F32)
    bc1 = np.asarray(bc1, F32)

    C = build_constants(W1, b1, Wsig, Wsig_d, Wc1, bc1, Wc2, Wc2_d)
    cst = pack_const_tile(C)
    X, D2 = host_prep(rays_o, rays_d, W1, b1, Wc1, bc1)

    in_maps = []
    for cidx in range(N_CORES):
        sl = slice(cidx * R_CORE, (cidx + 1) * R_CORE)
        in_maps.append({
            "x": np.ascontiguousarray(X[:, sl]),
            "d2": np.ascontiguousarray(D2[:, sl]),
            "cst": cst,
        })

    if "nc" not in _CACHED:
        _CACHED["nc"] = _build_module()
    nc = _CACHED["nc"]

    results = run_bass_kernel_spmd(nc, in_maps, core_ids=list(range(N_CORES)))
    out = np.concatenate(
        [results.results[cidx]["y"].T for cidx in range(N_CORES)], axis=0)
    return np.ascontiguousarray(out.astype(np.float32))


if __name__ == "__main__":
    rng = np.random.default_rng(0)
    ins = {
        "rays_o": (rng.random((N_RAYS, 3), dtype=np.float32) - 0.5),
        "rays_d": rng.standard_normal((N_RAYS, 3)).astype(np.float32),
        "W1": rng.standard_normal((3, 32)).astype(np.float32) * 0.5,
        "b1": np.zeros((32,), np.float32),
        "Wsig": rng.standard_normal((32, 1)).astype(np.float32) * 0.5,
        "Wsig_d": rng.standard_normal((32, 1)).astype(np.float32) * 0.5,
        "Wc1": rng.standard_normal((6, 32)).astype(np.float32) * 0.5,
        "bc1": np.zeros((32,), np.float32),
        "Wc2": rng.standard_normal((32, 3)).astype(np.float32) * 0.5,
        "Wc2_d": rng.standard_normal((32, 3)).astype(np.float32) * 0.5,
        "num_steps": 128,
    }
    out = kernel(**ins)
    print("out", out.shape, out.dtype, np.isfinite(out).all())


# revision 6
# speedup vs baseline: 16.1430x; 1.3215x over previous
"""NeRF renderer on 8 Trainium2 NeuronCores (Bass/Tile).

kernel(**inputs) takes FULL inputs (rays_o/rays_d [32768,3], MLP params,
num_steps=128) and returns the FULL [32768,9] output. Rays are sharded 8 ways
(4096 rays/core); params are replicated (baked into per-core constants).

Math: per ray, pre-activation hiddens are linear in z (H = P + z_t*Q), so the
host precomputes per-ray P/Q/Pc/Qc (and AABB near/far -> deltas). The device
evaluates relu/heads via small matmuls packed t-on-partition in PSUM, then
composites with a triangular-matmul cumsum and telescoped weights
w = (1-exp(-x)) * exp(x-S). softplus/sigmoid are built from exp+ln so the
whole kernel uses one ScalarE table set.
"""

import sys
from contextlib import ExitStack

for _p in ("/opt/trn_rl_repo", "/root/.axon_site/_ro/trn_rl_repo"):
    if _p not in sys.path:
        sys.path.insert(0, _p)

import numpy as np

N_CORES = 8
N_RAYS = 32768
R_CORE = N_RAYS // N_CORES
RC = 512
T = 128
H = 32
F32 = np.float32

Z = (np.arange(T, dtype=np.float64) / (T - 1)).astype(F32)

CONST_COLS = dict(
    h=0, sig=4096, rgb=4224, tri=4256, sum0=4384, sel=4512, red=5024,
    wd=5030, ya=5038, yb=5047, yb1=5056, dl=5065, one=5321, total=5833,
)


def _sig_rho(ul, h2, g):
    return 32 * (ul & 3) + 8 * (ul >> 2) + 4 * h2 + g


def _rgb_rho(ul, g, c2):
    return 32 * ((ul + 2) & 3) + 6 * g + c2


def build_constants(W1, b1, Wsig, Wsig_d, Wc1, bc1, Wc2, Wc2_d):
    C = {}
    lhsT_H = np.zeros((32, 64, 128), F32)
    for u in range(32):
        for g in range(4):
            for j in range(H):
                lhsT_H[u, j, 32 * g + j] = 1.0
                lhsT_H[u, H + j, 32 * g + j] = Z[4 * u + g]
    C["lhsT_H"] = lhsT_H

    Wsig2 = [np.asarray(Wsig, F32)[:, 0], np.asarray(Wsig_d, F32)[:, 0]]
    lhsT_sig = np.zeros((4, 128, 32), F32)
    for qp in range(4):
        for g in range(4):
            for h2 in range(2):
                for j in range(H):
                    lhsT_sig[qp, 32 * g + j, 8 * qp + 4 * h2 + g] = Wsig2[h2][j]
    C["lhsT_sig"] = lhsT_sig

    Wc2all = np.concatenate([np.asarray(Wc2, F32), np.asarray(Wc2_d, F32)], axis=1)
    lhsT_rgb = np.zeros((128, 32), F32)
    for g in range(4):
        for c2 in range(6):
            for j in range(H):
                lhsT_rgb[32 * g + j, 6 * g + c2] = Wc2all[j, c2]
    C["lhsT_rgb"] = lhsT_rgb

    rho_t = np.zeros(128, np.int64)
    rho_h2 = np.zeros(128, np.int64)
    for ul in range(16):
        for h2 in range(2):
            for g in range(4):
                rho = _sig_rho(ul, h2, g)
                rho_t[rho] = 4 * ul + g
                rho_h2[rho] = h2
    C["lhsT_tri"] = ((rho_h2[:, None] == rho_h2[None, :])
                     & (rho_t[:, None] <= rho_t[None, :])).astype(F32)
    C["lhsT_sum0"] = (rho_h2[:, None] == rho_h2[None, :]).astype(F32)

    lhsT_sel = np.zeros((4, 128, 128), F32)
    for beta in range(4):
        for ul in range(4 * beta, 4 * beta + 4):
            for g in range(4):
                for c2 in range(6):
                    rr = _rgb_rho(ul, g, c2)
                    h2 = 1 if c2 >= 3 else 0
                    src = np.where((rho_t == 4 * ul + g) & (rho_h2 == h2))[0]
                    lhsT_sel[beta, src[0], rr] = 1.0
    C["lhsT_sel"] = lhsT_sel

    lhsT_red = np.zeros((128, 6), F32)
    for rr in range(128):
        c24 = rr & 31
        if c24 < 24:
            lhsT_red[rr, c24 % 6] = 1.0
    C["lhsT_red"] = lhsT_red

    lhsT_wd = np.zeros((2, 128, 4), F32)
    for seg in range(2):
        for rho in range(128):
            h2 = rho_h2[rho]
            lhsT_wd[seg, rho, 2 * h2 + 0] = 1.0
            lhsT_wd[seg, rho, 2 * h2 + 1] = Z[64 * seg + rho_t[rho]]
    C["lhsT_wd"] = lhsT_wd

    lhsT_ya = np.zeros((6, 9), F32)
    for c2 in range(6):
        lhsT_ya[c2, c2 if c2 < 3 else 2 + c2] = 1.0
    C["lhsT_ya"] = lhsT_ya

    yb = np.zeros((5, 9), F32)
    yb[0, 0:3] = -1.0
    yb[0, 4] = 1.0
    yb[1, 3] = 1.0
    yb[2, 5:8] = -1.0
    yb[3, 8] = 1.0
    yb[4, 0:3] = 1.0
    yb[4, 5:8] = 1.0
    C["lhsT_yb"] = yb

    lhsT_dl = np.zeros((2, 2, 128), F32)
    for seg in range(2):
        for rho in range(128):
            tg = 64 * seg + rho_t[rho]
            lhsT_dl[seg, 0 if tg != 127 else 1, rho] = 1.0
    C["lhsT_dl"] = lhsT_dl
    return C


def pack_const_tile(C):
    CC = CONST_COLS
    ct = np.zeros((128, CC["total"]), F32)
    for u in range(32):
        ct[0:64, 128 * u:128 * u + 128] = C["lhsT_H"][u]
        ct[64:128, 128 * u:128 * u + 128] = C["lhsT_H"][u]
    for qp in range(4):
        ct[:, CC["sig"] + 32 * qp:CC["sig"] + 32 * qp + 32] = C["lhsT_sig"][qp]
    ct[:, CC["rgb"]:CC["rgb"] + 32] = C["lhsT_rgb"]
    ct[:, CC["tri"]:CC["tri"] + 128] = C["lhsT_tri"]
    ct[:, CC["sum0"]:CC["sum0"] + 128] = C["lhsT_sum0"]
    for b in range(4):
        ct[:, CC["sel"] + 128 * b:CC["sel"] + 128 * b + 128] = C["lhsT_sel"][b]
    ct[:, CC["red"]:CC["red"] + 6] = C["lhsT_red"]
    for seg in range(2):
        ct[:, CC["wd"] + 4 * seg:CC["wd"] + 4 * seg + 4] = C["lhsT_wd"][seg]
    ct[0:6, CC["ya"]:CC["ya"] + 9] = C["lhsT_ya"]
    ct[0:4, CC["yb"]:CC["yb"] + 9] = C["lhsT_yb"][0:4]
    ct[0:1, CC["yb1"]:CC["yb1"] + 9] = C["lhsT_yb"][4:5]
    for seg in range(2):
        ct[0:2, CC["dl"] + 128 * seg:CC["dl"] + 128 * seg + 128] = C["lhsT_dl"][seg]
    ct[0:1, CC["one"]:CC["one"] + 512] = 1.0
    return ct


def host_prep(rays_o, rays_d, W1, b1, Wc1, bc1):
    o = np.asarray(rays_o, F32)
    rd = np.asarray(rays_d, F32)
    d = rd / np.linalg.norm(rd.astype(np.float64), axis=-1, keepdims=True).astype(F32)
    inv = 1.0 / d
    t1 = (-1.0 - o) * inv
    t2 = (1.0 - o) * inv
    near = np.maximum(np.minimum(t1, t2).max(-1), F32(0.2))
    far = np.maximum(np.maximum(t1, t2).min(-1), near + F32(1e-6))
    span = far - near
    A = o + d * near[:, None]
    B = d * span[:, None]
    P = A @ W1 + b1
    Q = B @ W1
    Pc = A @ Wc1[:3] + d @ Wc1[3:] + bc1
    Qc = B @ Wc1[:3]
    X = np.concatenate([P.T, Q.T, Pc.T, Qc.T], axis=0).astype(F32)
    D2 = np.stack([span / F32(T - 1), span / F32(T)], axis=0).astype(F32)
    return np.ascontiguousarray(X), np.ascontiguousarray(D2)


def emit_nerf(tc, y_ap, x_ap, d2_ap, cst_ap, n_rays=R_CORE):
    import concourse.mybir as mybir
    AF = mybir.ActivationFunctionType
    ALU = mybir.AluOpType
    f32 = mybir.dt.float32
    nc = tc.nc
    nchunk = n_rays // RC
    CC = CONST_COLS

    with ExitStack() as ctx:
        singles = ctx.enter_context(tc.tile_pool(name="singles", bufs=1))
        xpool = ctx.enter_context(tc.tile_pool(name="xpool", bufs=2))
        hpool = ctx.enter_context(tc.tile_pool(name="hpool", bufs=2))
        cpool = ctx.enter_context(tc.tile_pool(name="cpool", bufs=2))
        rgbpool = ctx.enter_context(tc.tile_pool(name="rgbpool", bufs=8))
        opool = ctx.enter_context(tc.tile_pool(name="opool", bufs=2))
        psH = ctx.enter_context(tc.tile_pool(name="psH", bufs=1, space="PSUM"))
        psHC = ctx.enter_context(tc.tile_pool(name="psHC", bufs=1, space="PSUM"))
        psSig = ctx.enter_context(tc.tile_pool(name="psSig", bufs=2, space="PSUM"))
        psRgb = ctx.enter_context(tc.tile_pool(name="psRgb", bufs=4, space="PSUM"))

        cst = singles.tile([128, CC["total"]], f32)
        nc.sync.dma_start(out=cst[:], in_=cst_ap[:])

        def cs(key, off, k, w):
            c0 = CC[key] + off
            return cst[0:k, c0:c0 + w] if k != 128 else cst[:, c0:c0 + w]

        for c in range(nchunk):
            x_c = xpool.tile([128, RC], f32, tag="xc", name=f"xc{c}")
            nc.sync.dma_start(out=x_c[:], in_=x_ap[:, c * RC:(c + 1) * RC])
            d2_c = xpool.tile([2, RC], f32, tag="d2c", name=f"d2c{c}")
            nc.sync.dma_start(out=d2_c[:], in_=d2_ap[:, c * RC:(c + 1) * RC])

            x_sb = [None, None]
            rgb_sb = [[None] * 4, [None] * 4]
            w_sb = [None, None]

            for seg in range(2):
                sig_ps = psSig.tile([128, RC], f32, tag="sig", name=f"sig{c}_{seg}")
                rgb_ps = [None] * 4
                for ul in range(16):
                    u = 16 * seg + ul
                    hps = psH.tile([128, RC], f32, tag="h", name=f"h{c}_{u}")
                    nc.tensor.matmul(
                        hps[:], cst[0:64, 128 * u:128 * (u + 1)], x_c[0:64, :],
                        start=True, stop=True)
                    hcps = psHC.tile([128, RC], f32, tag="hc", name=f"hc{c}_{u}")
                    nc.tensor.matmul(
                        hcps[:], cst[64:128, 128 * u:128 * (u + 1)], x_c[64:128, :],
                        start=True, stop=True)
                    h_sb = hpool.tile([128, RC], f32, tag="hsb", name=f"hsb{c}_{u}")
                    nc.scalar.activation(h_sb[:], hps[:], AF.Relu)
                    hc_sb = hpool.tile([128, RC], f32, tag="hcsb", name=f"hcsb{c}_{u}")
                    nc.vector.tensor_scalar_max(hc_sb[:], hcps[:], 0.0)

                    s = ul & 3
                    qp = ul >> 2
                    nc.tensor.matmul(
                        sig_ps[32 * s:32 * s + 32, :],
                        cs("sig", 32 * qp, 128, 32), h_sb[:],
                        start=(qp == 0), stop=(qp == 3),
                        tile_position=(0, 32 * s), skip_group_check=True)
                    sr = (ul + 2) & 3
                    beta = ul >> 2
                    if rgb_ps[beta] is None:
                        rgb_ps[beta] = psRgb.tile([128, RC], f32, tag="rgb",
                                                  name=f"rgbps{c}_{seg}_{beta}")
                    nc.tensor.matmul(
                        rgb_ps[beta][32 * sr:32 * sr + 32, :],
                        cs("rgb", 0, 128, 32), hc_sb[:],
                        start=True, stop=True,
                        tile_position=(0, 32 * sr), skip_group_check=True)

                a_sb = cpool.tile([128, RC], f32, tag="a", name=f"a{c}_{seg}")
                nc.scalar.activation(a_sb[:], sig_ps[:], AF.Exp)
                L_sb = cpool.tile([128, RC], f32, tag="L", name=f"L{c}_{seg}")
                nc.scalar.activation(L_sb[:], a_sb[:], AF.Ln, bias=1.0)
                dl_ps = psH.tile([128, RC], f32, tag="h", name=f"dl{c}_{seg}")
                nc.tensor.matmul(
                    dl_ps[:], cs("dl", 128 * seg, 2, 128), d2_c[:],
                    start=True, stop=True)
                xs = cpool.tile([128, RC], f32, tag="x", name=f"x{c}_{seg}")
                nc.vector.tensor_tensor(xs[:], L_sb[:], dl_ps[:], op=ALU.mult)
                x_sb[seg] = xs

                for beta in range(4):
                    m_sb = cpool.tile([128, RC], f32, tag="m", name=f"m{c}_{seg}_{beta}")
                    nc.scalar.activation(m_sb[:], rgb_ps[beta][:], AF.Exp, scale=-1.0)
                    p_sb = cpool.tile([128, RC], f32, tag="p", name=f"p{c}_{seg}_{beta}")
                    nc.scalar.activation(p_sb[:], m_sb[:], AF.Ln, bias=1.0)
                    r_sb = rgbpool.tile([128, RC], f32, tag="rgbsb",
                                        name=f"rgbsb{c}_{seg}_{beta}")
                    nc.scalar.activation(r_sb[:], p_sb[:], AF.Exp, scale=-1.0)
                    rgb_sb[seg][beta] = r_sb

            for seg in range(2):
                S_ps = psHC.tile([128, RC], f32, tag="hc", name=f"S{c}_{seg}")
                nc.tensor.matmul(S_ps[:], cs("tri", 0, 128, 128), x_sb[seg][:],
                                 start=True, stop=(seg == 0))
                if seg == 1:
                    nc.tensor.matmul(S_ps[:], cs("sum0", 0, 128, 128), x_sb[0][:],
                                     start=False, stop=True)
                tmp = cpool.tile([128, RC], f32, tag="tmp", name=f"tmp{c}_{seg}")
                nc.vector.tensor_tensor(tmp[:], x_sb[seg][:], S_ps[:], op=ALU.subtract)
                E_sb = cpool.tile([128, RC], f32, tag="E", name=f"E{c}_{seg}")
                nc.scalar.activation(E_sb[:], tmp[:], AF.Exp)
                y1_sb = cpool.tile([128, RC], f32, tag="y1", name=f"y1{c}_{seg}")
                nc.scalar.activation(y1_sb[:], x_sb[seg][:], AF.Exp, scale=-1.0)
                t2 = cpool.tile([128, RC], f32, tag="t2", name=f"t2{c}_{seg}")
                nc.vector.tensor_scalar(t2[:], y1_sb[:], -1.0, 1.0,
                                        op0=ALU.mult, op1=ALU.add)
                ws_ = cpool.tile([128, RC], f32, tag="w", name=f"w{c}_{seg}")
                nc.vector.tensor_tensor(ws_[:], t2[:], E_sb[:], op=ALU.mult)
                w_sb[seg] = ws_

            img_ps = psRgb.tile([6, RC], f32, tag="rgb", name=f"img{c}")
            n_img = 0
            for seg in range(2):
                for beta in range(4):
                    wrep_ps = psSig.tile([128, RC], f32, tag="sig",
                                         name=f"wrep{c}_{seg}_{beta}")
                    nc.tensor.matmul(wrep_ps[:], cs("sel", 128 * beta, 128, 128),
                                     w_sb[seg][:], start=True, stop=True)
                    wrgb = cpool.tile([128, RC], f32, tag="wrgb",
                                      name=f"wrgb{c}_{seg}_{beta}")
                    nc.vector.tensor_tensor(wrgb[:], rgb_sb[seg][beta][:],
                                            wrep_ps[:], op=ALU.mult)
                    nc.tensor.matmul(img_ps[:], cs("red", 0, 128, 6), wrgb[:],
                                     start=(n_img == 0), stop=(n_img == 7),
                                     skip_group_check=True)
                    n_img += 1

            wd_ps = psRgb.tile([4, RC], f32, tag="rgb", name=f"wd{c}")
            nc.tensor.matmul(wd_ps[:], cs("wd", 0, 128, 4), w_sb[0][:],
                             start=True, stop=False)
            nc.tensor.matmul(wd_ps[:], cs("wd", 4, 128, 4), w_sb[1][:],
                             start=False, stop=True)

            img_sb = opool.tile([6, RC], f32, tag="img", name=f"imgsb{c}")
            nc.scalar.activation(img_sb[:], img_ps[:], AF.Copy)
            wd_sb = opool.tile([4, RC], f32, tag="wd", name=f"wdsb{c}")
            nc.scalar.activation(wd_sb[:], wd_ps[:], AF.Copy)

            y_ps = psRgb.tile([9, RC], f32, tag="rgb", name=f"y{c}")
            nc.tensor.matmul(y_ps[:], cs("ya", 0, 6, 9), img_sb[:],
                             start=True, stop=False)
            nc.tensor.matmul(y_ps[:], cs("yb", 0, 4, 9), wd_sb[:],
                             start=False, stop=False)
            nc.tensor.matmul(y_ps[:], cs("yb1", 0, 1, 9), cs("one", 0, 1, RC),
                             start=False, stop=True)
            y_sb = opool.tile([9, RC], f32, tag="ysb", name=f"ysb{c}")
            nc.scalar.activation(y_sb[:], y_ps[:], AF.Copy)
            nc.sync.dma_start(out=y_ap[:, c * RC:(c + 1) * RC], in_=y_sb[:])


_CACHED = {}


def _build_runner(nc):
    """Persistent jitted SPMD runner (avoids bass2jax's per-call re-jit)."""
    import jax
    from jax.sharding import Mesh, PartitionSpec
    from jax.experimental.shard_map import shard_map
    from concourse import bass2jax

    bass2jax.install_neuronx_cc_hook()
    in_names = ["x", "d2", "cst"]
    out_names = ["y"]
    out_avals = [jax.core.ShapedArray((9, R_CORE), np.float32)]
    all_names = in_names + out_names
    pname = nc.partition_id_tensor.name if nc.partition_id_tensor else None
    if pname is not None:
        all_names = all_names + [pname]

    def _body(*args):
        operands = list(args)
        if pname is not None:
            operands.append(bass2jax.partition_id_tensor())
        outs = bass2jax._bass_exec_p.bind(
            *operands,
            out_avals=tuple(out_avals),
            in_names=tuple(all_names),
            out_names=tuple(out_names),
            lowering_input_output_aliases=(),
            sim_require_finite=True,
            sim_require_nnan=True,
            nc=nc,
        )
        return tuple(outs)

    devices = jax.devices()[:N_CORES]
    mesh = Mesh(np.asarray(devices), ("core",))
    n_in = len(in_names) + 1  # + donated zero output
    sharded = jax.jit(
        shard_map(_body, mesh=mesh,
                  in_specs=(PartitionSpec("core"),) * n_in,
                  out_specs=(PartitionSpec("core"),) * len(out_names),
                  check_rep=False),
        donate_argnums=(len(in_names),), keep_unused=True)

    def run(in_maps):
        concat = [np.concatenate([m[k] for m in in_maps], axis=0)
                  for k in in_names]
        zeros = np.zeros((N_CORES * 9, R_CORE), np.float32)
        (y_out,) = sharded(*concat, zeros)
        return np.asarray(y_out).reshape(N_CORES, 9, R_CORE)

    return run


def _build_module():
    import concourse.bacc as bacc
    import concourse.tile as tile
    import concourse.mybir as mybir

    nc = bacc.Bacc("TRN2", target_bir_lowering=False, debug=False)
    x = nc.dram_tensor("x", [128, R_CORE], mybir.dt.float32, kind="ExternalInput")
    d2 = nc.dram_tensor("d2", [2, R_CORE], mybir.dt.float32, kind="ExternalInput")
    cst = nc.dram_tensor("cst", [128, CONST_COLS["total"]], mybir.dt.float32,
                         kind="ExternalInput")
    y = nc.dram_tensor("y", [9, R_CORE], mybir.dt.float32, kind="ExternalOutput")
    with tile.TileContext(nc) as tc:
        emit_nerf(tc, y.ap(), x.ap(), d2.ap(), cst.ap(), n_rays=R_CORE)
    nc.compile()
    return nc


def kernel(rays_o, rays_d, W1, b1, Wsig, Wsig_d, Wc1, bc1, Wc2, Wc2_d, num_steps):
    assert int(num_steps) == T
    W1 = np.asarray(W1, F32)
    b1 = np.asarray(b1, F32)
    Wc1 = np.asarray(Wc1, F32)
    bc1 = np.asarray(bc1, F32)

    C = build_constants(W1, b1, Wsig, Wsig_d, Wc1, bc1, Wc2, Wc2_d)
    cst = pack_const_tile(C)
    X, D2 = host_prep(rays_o, rays_d, W1, b1, Wc1, bc1)

    in_maps = []
    for cidx in range(N_CORES):
        sl = slice(cidx * R_CORE, (cidx + 1) * R_CORE)
        in_maps.append({
            "x": np.ascontiguousarray(X[:, sl]),
            "d2": np.ascontiguousarray(D2[:, sl]),
            "cst": cst,
        })

    if "run" not in _CACHED:
        _CACHED["nc"] = _build_module()
        _CACHED["run"] = _build_runner(_CACHED["nc"])

    y = _CACHED["run"](in_maps)  # [N_CORES, 9, R_CORE]
    out = np.concatenate([y[cidx].T for cidx in range(N_CORES)], axis=0)
    return np.ascontiguousarray(out.astype(np.float32))


if __name__ == "__main__":
    rng = np.random.default_rng(0)
    ins = {
        "rays_o": (rng.random((N_RAYS, 3), dtype=np.float32) - 0.5),
        "rays_d": rng.standard_normal((N_RAYS, 3)).astype(np.float32),
        "W1": rng.standard_normal((3, 32)).astype(np.float32) * 0.5,
        "b1": np.zeros((32,), np.float32),
        "Wsig": rng.standard_normal((32, 1)).astype(np.float32) * 0.5,
        "Wsig_d": rng.standard_normal((32, 1)).astype(np.float32) * 0.5,
        "Wc1": rng.standard_normal((6, 32)).astype(np.float32) * 0.5,
        "bc1": np.zeros((32,), np.float32),
        "Wc2": rng.standard_normal((32, 3)).astype(np.float32) * 0.5,
        "Wc2_d": rng.standard_normal((32, 3)).astype(np.float32) * 0.5,
        "num_steps": 128,
    }
    out = kernel(**ins)
    print("out", out.shape, out.dtype, np.isfinite(out).all())


# revision 15
# speedup vs baseline: 131.2670x; 8.1315x over previous
"""NeRF renderer on 8 Trainium2 NeuronCores (Bass/Tile).

kernel(**inputs) takes FULL inputs (rays_o/rays_d [32768,3], MLP params,
num_steps=128) and returns the FULL [32768,9] output. Rays are sharded 8 ways
(4096 rays/core); params are replicated (baked into per-core constants).

Math: per ray, pre-activation hiddens are linear in z (H = P + z_t*Q), so the
host precomputes per-ray P/Q/Pc/Qc (and AABB near/far -> deltas). The device
evaluates relu/heads via small matmuls packed t-on-partition in PSUM, then
composites with a triangular-matmul cumsum and telescoped weights
w = (1-exp(-x)) * exp(x-S). softplus/sigmoid are built from exp+ln so the
whole kernel uses one ScalarE table set.
"""

import sys
from contextlib import ExitStack

for _p in ("/opt/trn_rl_repo", "/root/.axon_site/_ro/trn_rl_repo"):
    if _p not in sys.path:
        sys.path.insert(0, _p)

import numpy as np

N_CORES = 8
N_RAYS = 32768
R_CORE = N_RAYS // N_CORES
RC = 512
T = 128
H = 32
F32 = np.float32

Z = (np.arange(T, dtype=np.float64) / (T - 1)).astype(F32)

CONST_COLS = dict(
    h=0, sig=4096, rgb=4224, tri=4256, sum0=4384, sel=4512, red=5024,
    wd=5030, ya=5038, yb=5047, yb1=5056, dl=5065, one=5321, pq=5833,
    total=5961,
)


def _sig_rho(ul, h2, g):
    return 32 * (ul & 3) + 8 * (ul >> 2) + 4 * h2 + g


def _rgb_rho(ul, g, c2):
    return 32 * ((ul + 2) & 3) + 6 * g + c2


def build_constants(W1, b1, Wsig, Wsig_d, Wc1, bc1, Wc2, Wc2_d):
    C = {}
    lhsT_H = np.zeros((32, 64, 128), F32)
    for u in range(32):
        for g in range(4):
            for j in range(H):
                lhsT_H[u, j, 32 * g + j] = 1.0
                lhsT_H[u, H + j, 32 * g + j] = Z[4 * u + g]
    C["lhsT_H"] = lhsT_H

    Wsig2 = [np.asarray(Wsig, F32)[:, 0], np.asarray(Wsig_d, F32)[:, 0]]
    lhsT_sig = np.zeros((4, 128, 32), F32)
    for qp in range(4):
        for g in range(4):
            for h2 in range(2):
                for j in range(H):
                    lhsT_sig[qp, 32 * g + j, 8 * qp + 4 * h2 + g] = Wsig2[h2][j]
    C["lhsT_sig"] = lhsT_sig

    Wc2all = np.concatenate([np.asarray(Wc2, F32), np.asarray(Wc2_d, F32)], axis=1)
    lhsT_rgb = np.zeros((128, 32), F32)
    for g in range(4):
        for c2 in range(6):
            for j in range(H):
                lhsT_rgb[32 * g + j, 6 * g + c2] = Wc2all[j, c2]
    C["lhsT_rgb"] = lhsT_rgb

    rho_t = np.zeros(128, np.int64)
    rho_h2 = np.zeros(128, np.int64)
    for ul in range(16):
        for h2 in range(2):
            for g in range(4):
                rho = _sig_rho(ul, h2, g)
                rho_t[rho] = 4 * ul + g
                rho_h2[rho] = h2
    C["lhsT_tri"] = ((rho_h2[:, None] == rho_h2[None, :])
                     & (rho_t[:, None] <= rho_t[None, :])).astype(F32)
    C["lhsT_sum0"] = (rho_h2[:, None] == rho_h2[None, :]).astype(F32)

    lhsT_sel = np.zeros((4, 128, 128), F32)
    for beta in range(4):
        for ul in range(4 * beta, 4 * beta + 4):
            for g in range(4):
                for c2 in range(6):
                    rr = _rgb_rho(ul, g, c2)
                    h2 = 1 if c2 >= 3 else 0
                    src = np.where((rho_t == 4 * ul + g) & (rho_h2 == h2))[0]
                    lhsT_sel[beta, src[0], rr] = 1.0
    C["lhsT_sel"] = lhsT_sel

    lhsT_red = np.zeros((128, 6), F32)
    for rr in range(128):
        c24 = rr & 31
        if c24 < 24:
            lhsT_red[rr, c24 % 6] = 1.0
    C["lhsT_red"] = lhsT_red

    lhsT_wd = np.zeros((2, 128, 4), F32)
    for seg in range(2):
        for rho in range(128):
            h2 = rho_h2[rho]
            lhsT_wd[seg, rho, 2 * h2 + 0] = 1.0
            lhsT_wd[seg, rho, 2 * h2 + 1] = Z[64 * seg + rho_t[rho]]
    C["lhsT_wd"] = lhsT_wd

    lhsT_ya = np.zeros((6, 9), F32)
    for c2 in range(6):
        lhsT_ya[c2, c2 if c2 < 3 else 2 + c2] = 1.0
    C["lhsT_ya"] = lhsT_ya

    yb = np.zeros((5, 9), F32)
    yb[0, 0:3] = -1.0
    yb[0, 4] = 1.0
    yb[1, 3] = 1.0
    yb[2, 5:8] = -1.0
    yb[3, 8] = 1.0
    yb[4, 0:3] = 1.0
    yb[4, 5:8] = 1.0
    C["lhsT_yb"] = yb

    lhsT_dl = np.zeros((2, 2, 128), F32)
    for seg in range(2):
        for rho in range(128):
            tg = 64 * seg + rho_t[rho]
            lhsT_dl[seg, 0 if tg != 127 else 1, rho] = 1.0
    C["lhsT_dl"] = lhsT_dl

    # on-device P/Q/Pc/Qc build: [10, 128] from rows (A3, B3, d3, ones)
    W1 = np.asarray(W1, F32)
    b1 = np.asarray(b1, F32)
    Wc1 = np.asarray(Wc1, F32)
    bc1 = np.asarray(bc1, F32)
    pq = np.zeros((10, 128), F32)
    for j in range(H):
        for ci in range(3):
            pq[ci, j] = W1[ci, j]            # P
            pq[3 + ci, 32 + j] = W1[ci, j]   # Q
            pq[ci, 64 + j] = Wc1[ci, j]      # Pc (A part)
            pq[6 + ci, 64 + j] = Wc1[3 + ci, j]  # Pc (d part)
            pq[3 + ci, 96 + j] = Wc1[ci, j]  # Qc
        pq[9, j] = b1[j]
        pq[9, 64 + j] = bc1[j]
    C["lhsT_pq"] = pq
    return C


def pack_const_tile(C):
    CC = CONST_COLS
    ct = np.zeros((128, CC["total"]), F32)
    for u in range(32):
        ct[0:64, 128 * u:128 * u + 128] = C["lhsT_H"][u]
        ct[64:128, 128 * u:128 * u + 128] = C["lhsT_H"][u]
    for qp in range(4):
        ct[:, CC["sig"] + 32 * qp:CC["sig"] + 32 * qp + 32] = C["lhsT_sig"][qp]
    ct[:, CC["rgb"]:CC["rgb"] + 32] = C["lhsT_rgb"]
    ct[:, CC["tri"]:CC["tri"] + 128] = C["lhsT_tri"]
    ct[:, CC["sum0"]:CC["sum0"] + 128] = C["lhsT_sum0"]
    for b in range(4):
        ct[:, CC["sel"] + 128 * b:CC["sel"] + 128 * b + 128] = C["lhsT_sel"][b]
    ct[:, CC["red"]:CC["red"] + 6] = C["lhsT_red"]
    for seg in range(2):
        ct[:, CC["wd"] + 4 * seg:CC["wd"] + 4 * seg + 4] = C["lhsT_wd"][seg]
    ct[0:6, CC["ya"]:CC["ya"] + 9] = C["lhsT_ya"]
    ct[0:4, CC["yb"]:CC["yb"] + 9] = C["lhsT_yb"][0:4]
    ct[0:1, CC["yb1"]:CC["yb1"] + 9] = C["lhsT_yb"][4:5]
    for seg in range(2):
        ct[0:2, CC["dl"] + 128 * seg:CC["dl"] + 128 * seg + 128] = C["lhsT_dl"][seg]
    ct[0:1, CC["one"]:CC["one"] + 512] = 1.0
    ct[0:10, CC["pq"]:CC["pq"] + 128] = C["lhsT_pq"]
    return ct


def host_prep(rays_o, rays_d):
    """Per-ray prep -> R10 [10, N] rows (A3, B3, d3, ones), D2 [2, N]."""
    o = np.asarray(rays_o, F32)
    rd = np.asarray(rays_d, F32)
    n2 = rd[:, 0] * rd[:, 0] + rd[:, 1] * rd[:, 1] + rd[:, 2] * rd[:, 2]
    d = rd * (1.0 / np.sqrt(n2))[:, None]
    inv = 1.0 / d
    t1 = (-1.0 - o) * inv
    t2 = (1.0 - o) * inv
    near = np.maximum(np.minimum(t1, t2).max(-1), F32(0.2))
    far = np.maximum(np.maximum(t1, t2).min(-1), near + F32(1e-6))
    span = far - near
    A = o + d * near[:, None]
    B = d * span[:, None]
    N = o.shape[0]
    R10 = np.empty((10, N), F32)
    R10[0:3] = A.T
    R10[3:6] = B.T
    R10[6:9] = d.T
    R10[9] = 1.0
    D2 = np.empty((2, N), F32)
    D2[0] = span * (1.0 / (T - 1))
    D2[1] = span * (1.0 / T)
    return R10, D2


def emit_nerf(tc, y_ap, x_ap, d2_ap, cst_ap, n_rays=R_CORE):
    import concourse.mybir as mybir
    AF = mybir.ActivationFunctionType
    ALU = mybir.AluOpType
    f32 = mybir.dt.float32
    nc = tc.nc
    nchunk = n_rays // RC
    CC = CONST_COLS

    with ExitStack() as ctx:
        singles = ctx.enter_context(tc.tile_pool(name="singles", bufs=1))
        xpool = ctx.enter_context(tc.tile_pool(name="xpool", bufs=2))
        hpool = ctx.enter_context(tc.tile_pool(name="hpool", bufs=2))
        cpool = ctx.enter_context(tc.tile_pool(name="cpool", bufs=2))
        rgbpool = ctx.enter_context(tc.tile_pool(name="rgbpool", bufs=8))
        opool = ctx.enter_context(tc.tile_pool(name="opool", bufs=2))
        psH = ctx.enter_context(tc.tile_pool(name="psH", bufs=1, space="PSUM"))
        psHC = ctx.enter_context(tc.tile_pool(name="psHC", bufs=1, space="PSUM"))
        psSig = ctx.enter_context(tc.tile_pool(name="psSig", bufs=2, space="PSUM"))
        psRgb = ctx.enter_context(tc.tile_pool(name="psRgb", bufs=4, space="PSUM"))

        cst = singles.tile([128, CC["total"]], f32)
        nc.sync.dma_start(out=cst[:], in_=cst_ap[:])

        def cs(key, off, k, w):
            c0 = CC[key] + off
            return cst[0:k, c0:c0 + w] if k != 128 else cst[:, c0:c0 + w]

        for c in range(nchunk):
            r_c = xpool.tile([10, RC], f32, tag="rc", name=f"rc{c}")
            nc.sync.dma_start(out=r_c[:], in_=x_ap[:, c * RC:(c + 1) * RC])
            d2_c = xpool.tile([2, RC], f32, tag="d2c", name=f"d2c{c}")
            nc.sync.dma_start(out=d2_c[:], in_=d2_ap[:, c * RC:(c + 1) * RC])
            x_ps = psH.tile([128, RC], f32, tag="h", name=f"xps{c}")
            nc.tensor.matmul(x_ps[:], cs("pq", 0, 10, 128), r_c[:],
                             start=True, stop=True)
            x_c = xpool.tile([128, RC], f32, tag="xc", name=f"xc{c}")
            nc.scalar.activation(x_c[:], x_ps[:], AF.Copy)

            x_sb = [None, None]
            rgb_sb = [[None] * 4, [None] * 4]
            w_sb = [None, None]

            for seg in range(2):
                sig_ps = psSig.tile([128, RC], f32, tag="sig", name=f"sig{c}_{seg}")
                rgb_ps = [None] * 4
                for ul in range(16):
                    u = 16 * seg + ul
                    hps = psH.tile([128, RC], f32, tag="h", name=f"h{c}_{u}")
                    nc.tensor.matmul(
                        hps[:], cst[0:64, 128 * u:128 * (u + 1)], x_c[0:64, :],
                        start=True, stop=True)
                    hcps = psHC.tile([128, RC], f32, tag="hc", name=f"hc{c}_{u}")
                    nc.tensor.matmul(
                        hcps[:], cst[64:128, 128 * u:128 * (u + 1)], x_c[64:128, :],
                        start=True, stop=True)
                    h_sb = hpool.tile([128, RC], f32, tag="hsb", name=f"hsb{c}_{u}")
                    nc.scalar.activation(h_sb[:], hps[:], AF.Relu)
                    hc_sb = hpool.tile([128, RC], f32, tag="hcsb", name=f"hcsb{c}_{u}")
                    nc.vector.tensor_scalar_max(hc_sb[:], hcps[:], 0.0)

                    s = ul & 3
                    qp = ul >> 2
                    nc.tensor.matmul(
                        sig_ps[32 * s:32 * s + 32, :],
                        cs("sig", 32 * qp, 128, 32), h_sb[:],
                        start=(qp == 0), stop=(qp == 3),
                        tile_position=(0, 32 * s), skip_group_check=True)
                    sr = (ul + 2) & 3
                    beta = ul >> 2
                    if rgb_ps[beta] is None:
                        rgb_ps[beta] = psRgb.tile([128, RC], f32, tag="rgb",
                                                  name=f"rgbps{c}_{seg}_{beta}")
                    nc.tensor.matmul(
                        rgb_ps[beta][32 * sr:32 * sr + 32, :],
                        cs("rgb", 0, 128, 32), hc_sb[:],
                        start=True, stop=True,
                        tile_position=(0, 32 * sr), skip_group_check=True)

                a_sb = cpool.tile([128, RC], f32, tag="a", name=f"a{c}_{seg}")
                nc.scalar.activation(a_sb[:], sig_ps[:], AF.Exp)
                L_sb = cpool.tile([128, RC], f32, tag="L", name=f"L{c}_{seg}")
                nc.scalar.activation(L_sb[:], a_sb[:], AF.Ln, bias=1.0)
                dl_ps = psH.tile([128, RC], f32, tag="h", name=f"dl{c}_{seg}")
                nc.tensor.matmul(
                    dl_ps[:], cs("dl", 128 * seg, 2, 128), d2_c[:],
                    start=True, stop=True)
                xs = cpool.tile([128, RC], f32, tag="x", name=f"x{c}_{seg}")
                nc.vector.tensor_tensor(xs[:], L_sb[:], dl_ps[:], op=ALU.mult)
                x_sb[seg] = xs

                for beta in range(4):
                    m_sb = cpool.tile([128, RC], f32, tag="m", name=f"m{c}_{seg}_{beta}")
                    nc.scalar.activation(m_sb[:], rgb_ps[beta][:], AF.Exp, scale=-1.0)
                    p_sb = cpool.tile([128, RC], f32, tag="p", name=f"p{c}_{seg}_{beta}")
                    nc.scalar.activation(p_sb[:], m_sb[:], AF.Ln, bias=1.0)
                    r_sb = rgbpool.tile([128, RC], f32, tag="rgbsb",
                                        name=f"rgbsb{c}_{seg}_{beta}")
                    nc.scalar.activation(r_sb[:], p_sb[:], AF.Exp, scale=-1.0)
                    rgb_sb[seg][beta] = r_sb

            for seg in range(2):
                S_ps = psHC.tile([128, RC], f32, tag="hc", name=f"S{c}_{seg}")
                nc.tensor.matmul(S_ps[:], cs("tri", 0, 128, 128), x_sb[seg][:],
                                 start=True, stop=(seg == 0))
                if seg == 1:
                    nc.tensor.matmul(S_ps[:], cs("sum0", 0, 128, 128), x_sb[0][:],
                                     start=False, stop=True)
                tmp = cpool.tile([128, RC], f32, tag="tmp", name=f"tmp{c}_{seg}")
                nc.vector.tensor_tensor(tmp[:], x_sb[seg][:], S_ps[:], op=ALU.subtract)
                E_sb = cpool.tile([128, RC], f32, tag="E", name=f"E{c}_{seg}")
                nc.scalar.activation(E_sb[:], tmp[:], AF.Exp)
                y1_sb = cpool.tile([128, RC], f32, tag="y1", name=f"y1{c}_{seg}")
                nc.scalar.activation(y1_sb[:], x_sb[seg][:], AF.Exp, scale=-1.0)
                t2 = cpool.tile([128, RC], f32, tag="t2", name=f"t2{c}_{seg}")
                nc.vector.tensor_scalar(t2[:], y1_sb[:], -1.0, 1.0,
                                        op0=ALU.mult, op1=ALU.add)
                ws_ = cpool.tile([128, RC], f32, tag="w", name=f"w{c}_{seg}")
                nc.vector.tensor_tensor(ws_[:], t2[:], E_sb[:], op=ALU.mult)
                w_sb[seg] = ws_

            img_ps = psRgb.tile([6, RC], f32, tag="rgb", name=f"img{c}")
            n_img = 0
            for seg in range(2):
                for beta in range(4):
                    wrep_ps = psSig.tile([128, RC], f32, tag="sig",
                                         name=f"wrep{c}_{seg}_{beta}")
                    nc.tensor.matmul(wrep_ps[:], cs("sel", 128 * beta, 128, 128),
                                     w_sb[seg][:], start=True, stop=True)
                    wrgb = cpool.tile([128, RC], f32, tag="wrgb",
                                      name=f"wrgb{c}_{seg}_{beta}")
                    nc.vector.tensor_tensor(wrgb[:], rgb_sb[seg][beta][:],
                                            wrep_ps[:], op=ALU.mult)
                    nc.tensor.matmul(img_ps[:], cs("red", 0, 128, 6), wrgb[:],
                                     start=(n_img == 0), stop=(n_img == 7),
                                     skip_group_check=True)
                    n_img += 1

            wd_ps = psRgb.tile([4, RC], f32, tag="rgb", name=f"wd{c}")
            nc.tensor.matmul(wd_ps[:], cs("wd", 0, 128, 4), w_sb[0][:],
                             start=True, stop=False)
            nc.tensor.matmul(wd_ps[:], cs("wd", 4, 128, 4), w_sb[1][:],
                             start=False, stop=True)

            img_sb = opool.tile([6, RC], f32, tag="img", name=f"imgsb{c}")
            nc.scalar.activation(img_sb[:], img_ps[:], AF.Copy)
            wd_sb = opool.tile([4, RC], f32, tag="wd", name=f"wdsb{c}")
            nc.scalar.activation(wd_sb[:], wd_ps[:], AF.Copy)

            y_ps = psRgb.tile([9, RC], f32, tag="rgb", name=f"y{c}")
            nc.tensor.matmul(y_ps[:], cs("ya", 0, 6, 9), img_sb[:],
                             start=True, stop=False)
            nc.tensor.matmul(y_ps[:], cs("yb", 0, 4, 9), wd_sb[:],
                             start=False, stop=False)
            nc.tensor.matmul(y_ps[:], cs("yb1", 0, 1, 9), cs("one", 0, 1, RC),
                             start=False, stop=True)
            y_sb = opool.tile([9, RC], f32, tag="ysb", name=f"ysb{c}")
            nc.scalar.activation(y_sb[:], y_ps[:], AF.Copy)
            nc.sync.dma_start(out=y_ap[:, c * RC:(c + 1) * RC], in_=y_sb[:])


_CACHED = {}


def _build_runner(nc):
    """Persistent jitted SPMD runner (avoids bass2jax's per-call re-jit)."""
    import jax
    from jax.sharding import Mesh, PartitionSpec
    from jax.experimental.shard_map import shard_map
    from concourse import bass2jax

    bass2jax.install_neuronx_cc_hook()
    in_names = ["x", "d2", "cst"]
    out_names = ["y"]
    out_avals = [jax.core.ShapedArray((9, R_CORE), np.float32)]
    sh = None  # set below
    all_names = in_names + out_names
    pname = nc.partition_id_tensor.name if nc.partition_id_tensor else None
    if pname is not None:
        all_names = all_names + [pname]

    def _body(*args):
        operands = list(args)
        if pname is not None:
            operands.append(bass2jax.partition_id_tensor())
        outs = bass2jax._bass_exec_p.bind(
            *operands,
            out_avals=tuple(out_avals),
            in_names=tuple(all_names),
            out_names=tuple(out_names),
            lowering_input_output_aliases=(),
            sim_require_finite=True,
            sim_require_nnan=True,
            nc=nc,
        )
        return tuple(outs)

    import jax.numpy as jnp
    from jax.sharding import NamedSharding

    devices = jax.devices()[:N_CORES]
    mesh = Mesh(np.asarray(devices), ("core",))
    sh = NamedSharding(mesh, PartitionSpec("core"))
    n_in = len(in_names) + 1  # + donated zero output
    sharded = jax.jit(
        shard_map(_body, mesh=mesh,
                  in_specs=(PartitionSpec("core"),) * n_in,
                  out_specs=(PartitionSpec("core"),) * len(out_names),
                  check_rep=False),
        donate_argnums=(len(in_names),), keep_unused=True)
    zfn = jax.jit(lambda: jnp.zeros((N_CORES * 9, R_CORE), np.float32),
                  out_shardings=sh)
    cst_cache = {}

    def run(x_cat, d2_cat, cst, cst_key):
        if cst_key not in cst_cache:
            cst_cache.clear()
            cst_cache[cst_key] = jax.device_put(
                np.concatenate([cst] * N_CORES, axis=0), sh)
        (y_out,) = sharded(x_cat, d2_cat, cst_cache[cst_key], zfn())
        return np.asarray(y_out).reshape(N_CORES, 9, R_CORE)

    return run


def _build_module():
    import concourse.bacc as bacc
    import concourse.tile as tile
    import concourse.mybir as mybir

    nc = bacc.Bacc("TRN2", target_bir_lowering=False, debug=False)
    x = nc.dram_tensor("x", [10, R_CORE], mybir.dt.float32, kind="ExternalInput")
    d2 = nc.dram_tensor("d2", [2, R_CORE], mybir.dt.float32, kind="ExternalInput")
    cst = nc.dram_tensor("cst", [128, CONST_COLS["total"]], mybir.dt.float32,
                         kind="ExternalInput")
    y = nc.dram_tensor("y", [9, R_CORE], mybir.dt.float32, kind="ExternalOutput")
    with tile.TileContext(nc) as tc:
        emit_nerf(tc, y.ap(), x.ap(), d2.ap(), cst.ap(), n_rays=R_CORE)
    nc.compile()
    return nc


def kernel(rays_o, rays_d, W1, b1, Wsig, Wsig_d, Wc1, bc1, Wc2, Wc2_d, num_steps):
    import hashlib

    assert int(num_steps) == T
    weights = [np.ascontiguousarray(np.asarray(a, F32))
               for a in (W1, b1, Wsig, Wsig_d, Wc1, bc1, Wc2, Wc2_d)]
    key = hashlib.md5(b"".join(a.tobytes() for a in weights)).hexdigest()

    if _CACHED.get("cst_key") != key:
        C = build_constants(*weights)
        _CACHED["cst"] = pack_const_tile(C)
        _CACHED["cst_key"] = key
    cst = _CACHED["cst"]

    R10, D2 = host_prep(rays_o, rays_d)
    # concat over cores: [N_CORES*10, R_CORE] etc (shard_map splits on axis 0)
    x_cat = np.ascontiguousarray(
        R10.reshape(10, N_CORES, R_CORE).transpose(1, 0, 2).reshape(
            N_CORES * 10, R_CORE))
    d2_cat = np.ascontiguousarray(
        D2.reshape(2, N_CORES, R_CORE).transpose(1, 0, 2).reshape(
            N_CORES * 2, R_CORE))

    if "run" not in _CACHED:
        _CACHED["nc"] = _build_module()
        _CACHED["run"] = _build_runner(_CACHED["nc"])

    y = _CACHED["run"](x_cat, d2_cat, cst, key)  # [N_CORES, 9, R_CORE]
    out = np.concatenate([y[cidx].T for cidx in range(N_CORES)], axis=0)
    return np.ascontiguousarray(out.astype(np.float32))


if __name__ == "__main__":
    rng = np.random.default_rng(0)
    ins = {
        "rays_o": (rng.random((N_RAYS, 3), dtype=np.float32) - 0.5),
        "rays_d": rng.standard_normal((N_RAYS, 3)).astype(np.float32),
        "W1": rng.standard_normal((3, 32)).astype(np.float32) * 0.5,
        "b1": np.zeros((32,), np.float32),
        "Wsig": rng.standard_normal((32, 1)).astype(np.float32) * 0.5,
        "Wsig_d": rng.standard_normal((32, 1)).astype(np.float32) * 0.5,
        "Wc1": rng.standard_normal((6, 32)).astype(np.float32) * 0.5,
        "bc1": np.zeros((32,), np.float32),
        "Wc2": rng.standard_normal((32, 3)).astype(np.float32) * 0.5,
        "Wc2_d": rng.standard_normal((32, 3)).astype(np.float32) * 0.5,
        "num_steps": 128,
    }
    out = kernel(**ins)
    print("out", out.shape, out.dtype, np.isfinite(out).all())


# revision 23
# speedup vs baseline: 148.0765x; 1.1281x over previous
"""NeRF renderer on 8 Trainium2 NeuronCores (Bass/Tile).

kernel(**inputs) takes FULL inputs (rays_o/rays_d [32768,3], MLP params,
num_steps=128) and returns the FULL [32768,9] output. Rays are sharded 8 ways
(4096 rays/core); params are replicated (baked into per-core constants).

Math: per ray, pre-activation hiddens are linear in z (H = P + z_t*Q), so the
host precomputes per-ray P/Q/Pc/Qc (and AABB near/far -> deltas). The device
evaluates relu/heads via small matmuls packed t-on-partition in PSUM, then
composites with a triangular-matmul cumsum and telescoped weights
w = (1-exp(-x)) * exp(x-S). softplus/sigmoid are built from exp+ln so the
whole kernel uses one ScalarE table set.
"""

import sys
from contextlib import ExitStack

for _p in ("/opt/trn_rl_repo", "/root/.axon_site/_ro/trn_rl_repo"):
    if _p not in sys.path:
        sys.path.insert(0, _p)

import numpy as np

N_CORES = 8
N_RAYS = 32768
R_CORE = N_RAYS // N_CORES
RC = 512
T = 128
H = 32
F32 = np.float32

Z = (np.arange(T, dtype=np.float64) / (T - 1)).astype(F32)

CONST_COLS = dict(
    h=0, sig=4096, rgb=4224, tri=4256, sum0=4384, sel=4512, red=5024,
    wd=5030, ya=5038, yb=5047, yb1=5056, dl=5065, one=5321, pq=5833,
    total=5961,
)


def _sig_rho(ul, h2, g):
    return 32 * (ul & 3) + 8 * (ul >> 2) + 4 * h2 + g


def _rgb_rho(ul, g, c2):
    return 32 * ((ul + 2) & 3) + 6 * g + c2


def build_constants(W1, b1, Wsig, Wsig_d, Wc1, bc1, Wc2, Wc2_d):
    C = {}
    lhsT_H = np.zeros((32, 64, 128), F32)
    for u in range(32):
        for g in range(4):
            for j in range(H):
                lhsT_H[u, j, 32 * g + j] = 1.0
                lhsT_H[u, H + j, 32 * g + j] = Z[4 * u + g]
    C["lhsT_H"] = lhsT_H

    Wsig2 = [np.asarray(Wsig, F32)[:, 0], np.asarray(Wsig_d, F32)[:, 0]]
    lhsT_sig = np.zeros((4, 128, 32), F32)
    for qp in range(4):
        for g in range(4):
            for h2 in range(2):
                for j in range(H):
                    lhsT_sig[qp, 32 * g + j, 8 * qp + 4 * h2 + g] = Wsig2[h2][j]
    C["lhsT_sig"] = lhsT_sig

    Wc2all = np.concatenate([np.asarray(Wc2, F32), np.asarray(Wc2_d, F32)], axis=1)
    lhsT_rgb = np.zeros((128, 32), F32)
    for g in range(4):
        for c2 in range(6):
            for j in range(H):
                lhsT_rgb[32 * g + j, 6 * g + c2] = Wc2all[j, c2]
    C["lhsT_rgb"] = lhsT_rgb

    rho_t = np.zeros(128, np.int64)
    rho_h2 = np.zeros(128, np.int64)
    for ul in range(16):
        for h2 in range(2):
            for g in range(4):
                rho = _sig_rho(ul, h2, g)
                rho_t[rho] = 4 * ul + g
                rho_h2[rho] = h2
    C["lhsT_tri"] = ((rho_h2[:, None] == rho_h2[None, :])
                     & (rho_t[:, None] <= rho_t[None, :])).astype(F32)
    C["lhsT_sum0"] = (rho_h2[:, None] == rho_h2[None, :]).astype(F32)

    lhsT_sel = np.zeros((4, 128, 128), F32)
    for beta in range(4):
        for ul in range(4 * beta, 4 * beta + 4):
            for g in range(4):
                for c2 in range(6):
                    rr = _rgb_rho(ul, g, c2)
                    h2 = 1 if c2 >= 3 else 0
                    src = np.where((rho_t == 4 * ul + g) & (rho_h2 == h2))[0]
                    lhsT_sel[beta, src[0], rr] = 1.0
    C["lhsT_sel"] = lhsT_sel

    lhsT_red = np.zeros((128, 6), F32)
    for rr in range(128):
        c24 = rr & 31
        if c24 < 24:
            lhsT_red[rr, c24 % 6] = 1.0
    C["lhsT_red"] = lhsT_red

    lhsT_wd = np.zeros((2, 128, 4), F32)
    for seg in range(2):
        for rho in range(128):
            h2 = rho_h2[rho]
            lhsT_wd[seg, rho, 2 * h2 + 0] = 1.0
            lhsT_wd[seg, rho, 2 * h2 + 1] = Z[64 * seg + rho_t[rho]]
    C["lhsT_wd"] = lhsT_wd

    lhsT_ya = np.zeros((6, 9), F32)
    for c2 in range(6):
        lhsT_ya[c2, c2 if c2 < 3 else 2 + c2] = 1.0
    C["lhsT_ya"] = lhsT_ya

    yb = np.zeros((5, 9), F32)
    yb[0, 0:3] = -1.0
    yb[0, 4] = 1.0
    yb[1, 3] = 1.0
    yb[2, 5:8] = -1.0
    yb[3, 8] = 1.0
    yb[4, 0:3] = 1.0
    yb[4, 5:8] = 1.0
    C["lhsT_yb"] = yb

    lhsT_dl = np.zeros((2, 2, 128), F32)
    for seg in range(2):
        for rho in range(128):
            tg = 64 * seg + rho_t[rho]
            lhsT_dl[seg, 0 if tg != 127 else 1, rho] = 1.0
    C["lhsT_dl"] = lhsT_dl

    # on-device P/Q/Pc/Qc build: [10, 128] from rows (A3, B3, d3, ones)
    W1 = np.asarray(W1, F32)
    b1 = np.asarray(b1, F32)
    Wc1 = np.asarray(Wc1, F32)
    bc1 = np.asarray(bc1, F32)
    pq = np.zeros((10, 128), F32)
    for j in range(H):
        for ci in range(3):
            pq[ci, j] = W1[ci, j]            # P
            pq[3 + ci, 32 + j] = W1[ci, j]   # Q
            pq[ci, 64 + j] = Wc1[ci, j]      # Pc (A part)
            pq[6 + ci, 64 + j] = Wc1[3 + ci, j]  # Pc (d part)
            pq[3 + ci, 96 + j] = Wc1[ci, j]  # Qc
        pq[9, j] = b1[j]
        pq[9, 64 + j] = bc1[j]
    C["lhsT_pq"] = pq
    return C


def pack_const_tile(C):
    CC = CONST_COLS
    ct = np.zeros((128, CC["total"]), F32)
    for u in range(32):
        ct[0:64, 128 * u:128 * u + 128] = C["lhsT_H"][u]
        ct[64:128, 128 * u:128 * u + 128] = C["lhsT_H"][u]
    for qp in range(4):
        ct[:, CC["sig"] + 32 * qp:CC["sig"] + 32 * qp + 32] = C["lhsT_sig"][qp]
    ct[:, CC["rgb"]:CC["rgb"] + 32] = C["lhsT_rgb"]
    ct[:, CC["tri"]:CC["tri"] + 128] = C["lhsT_tri"]
    ct[:, CC["sum0"]:CC["sum0"] + 128] = C["lhsT_sum0"]
    for b in range(4):
        ct[:, CC["sel"] + 128 * b:CC["sel"] + 128 * b + 128] = C["lhsT_sel"][b]
    ct[:, CC["red"]:CC["red"] + 6] = C["lhsT_red"]
    for seg in range(2):
        ct[:, CC["wd"] + 4 * seg:CC["wd"] + 4 * seg + 4] = C["lhsT_wd"][seg]
    ct[0:6, CC["ya"]:CC["ya"] + 9] = C["lhsT_ya"]
    ct[0:4, CC["yb"]:CC["yb"] + 9] = C["lhsT_yb"][0:4]
    ct[0:1, CC["yb1"]:CC["yb1"] + 9] = C["lhsT_yb"][4:5]
    for seg in range(2):
        ct[0:2, CC["dl"] + 128 * seg:CC["dl"] + 128 * seg + 128] = C["lhsT_dl"][seg]
    ct[0:1, CC["one"]:CC["one"] + 512] = 1.0
    ct[0:10, CC["pq"]:CC["pq"] + 128] = C["lhsT_pq"]
    return ct


def host_prep(rays_o, rays_d):
    """Per-ray prep -> R10 [10, N] rows (A3, B3, d3, ones), D2 [2, N]."""
    o = np.asarray(rays_o, F32)
    rd = np.asarray(rays_d, F32)
    n2 = rd[:, 0] * rd[:, 0] + rd[:, 1] * rd[:, 1] + rd[:, 2] * rd[:, 2]
    d = rd * (1.0 / np.sqrt(n2))[:, None]
    inv = 1.0 / d
    t1 = (-1.0 - o) * inv
    t2 = (1.0 - o) * inv
    near = np.maximum(np.minimum(t1, t2).max(-1), F32(0.2))
    far = np.maximum(np.maximum(t1, t2).min(-1), near + F32(1e-6))
    span = far - near
    A = o + d * near[:, None]
    B = d * span[:, None]
    N = o.shape[0]
    R12 = np.empty((12, N), F32)
    R12[0:3] = A.T
    R12[3:6] = B.T
    R12[6:9] = d.T
    R12[9] = 1.0
    R12[10] = span * (1.0 / (T - 1))
    R12[11] = span * (1.0 / T)
    return R12


def emit_nerf(tc, y_ap, x_ap, d2_ap, cst_ap, n_rays=R_CORE):
    import concourse.mybir as mybir
    AF = mybir.ActivationFunctionType
    ALU = mybir.AluOpType
    f32 = mybir.dt.float32
    nc = tc.nc
    nchunk = n_rays // RC
    CC = CONST_COLS

    with ExitStack() as ctx:
        singles = ctx.enter_context(tc.tile_pool(name="singles", bufs=1))
        xpool = ctx.enter_context(tc.tile_pool(name="xpool", bufs=2))
        hpool = ctx.enter_context(tc.tile_pool(name="hpool", bufs=2))
        cpool = ctx.enter_context(tc.tile_pool(name="cpool", bufs=2))
        rgbpool = ctx.enter_context(tc.tile_pool(name="rgbpool", bufs=8))
        opool = ctx.enter_context(tc.tile_pool(name="opool", bufs=2))
        psH = ctx.enter_context(tc.tile_pool(name="psH", bufs=1, space="PSUM"))
        psHC = ctx.enter_context(tc.tile_pool(name="psHC", bufs=1, space="PSUM"))
        psSig = ctx.enter_context(tc.tile_pool(name="psSig", bufs=2, space="PSUM"))
        psRgb = ctx.enter_context(tc.tile_pool(name="psRgb", bufs=4, space="PSUM"))

        cst = singles.tile([128, CC["total"]], f32)
        nc.sync.dma_start(out=cst[:], in_=cst_ap[:])

        def cs(key, off, k, w):
            c0 = CC[key] + off
            return cst[0:k, c0:c0 + w] if k != 128 else cst[:, c0:c0 + w]

        for c in range(nchunk):
            r_c = xpool.tile([10, RC], f32, tag="rc", name=f"rc{c}")
            nc.sync.dma_start(out=r_c[:], in_=x_ap[0:10, c * RC:(c + 1) * RC])
            d2_c = xpool.tile([2, RC], f32, tag="d2c", name=f"d2c{c}")
            nc.sync.dma_start(out=d2_c[:], in_=x_ap[10:12, c * RC:(c + 1) * RC])
            x_ps = psH.tile([128, RC], f32, tag="h", name=f"xps{c}")
            nc.tensor.matmul(x_ps[:], cs("pq", 0, 10, 128), r_c[:],
                             start=True, stop=True)
            x_c = xpool.tile([128, RC], f32, tag="xc", name=f"xc{c}")
            nc.scalar.activation(x_c[:], x_ps[:], AF.Copy)

            x_sb = [None, None]
            rgb_sb = [[None] * 4, [None] * 4]
            w_sb = [None, None]

            for seg in range(2):
                sig_ps = psSig.tile([128, RC], f32, tag="sig", name=f"sig{c}_{seg}")
                rgb_ps = [None] * 4
                for ul in range(16):
                    u = 16 * seg + ul
                    hps = psH.tile([128, RC], f32, tag="h", name=f"h{c}_{u}")
                    nc.tensor.matmul(
                        hps[:], cst[0:64, 128 * u:128 * (u + 1)], x_c[0:64, :],
                        start=True, stop=True)
                    hcps = psHC.tile([128, RC], f32, tag="hc", name=f"hc{c}_{u}")
                    nc.tensor.matmul(
                        hcps[:], cst[64:128, 128 * u:128 * (u + 1)], x_c[64:128, :],
                        start=True, stop=True)
                    h_sb = hpool.tile([128, RC], f32, tag="hsb", name=f"hsb{c}_{u}")
                    nc.scalar.activation(h_sb[:], hps[:], AF.Relu)
                    hc_sb = hpool.tile([128, RC], f32, tag="hcsb", name=f"hcsb{c}_{u}")
                    nc.vector.tensor_scalar_max(hc_sb[:], hcps[:], 0.0)

                    s = ul & 3
                    qp = ul >> 2
                    nc.tensor.matmul(
                        sig_ps[32 * s:32 * s + 32, :],
                        cs("sig", 32 * qp, 128, 32), h_sb[:],
                        start=(qp == 0), stop=(qp == 3),
                        tile_position=(0, 32 * s), skip_group_check=True)
                    sr = (ul + 2) & 3
                    beta = ul >> 2
                    if rgb_ps[beta] is None:
                        rgb_ps[beta] = psRgb.tile([128, RC], f32, tag="rgb",
                                                  name=f"rgbps{c}_{seg}_{beta}")
                    nc.tensor.matmul(
                        rgb_ps[beta][32 * sr:32 * sr + 32, :],
                        cs("rgb", 0, 128, 32), hc_sb[:],
                        start=True, stop=True,
                        tile_position=(0, 32 * sr), skip_group_check=True)

                a_sb = cpool.tile([128, RC], f32, tag="a", name=f"a{c}_{seg}")
                nc.scalar.activation(a_sb[:], sig_ps[:], AF.Exp)
                L_sb = cpool.tile([128, RC], f32, tag="L", name=f"L{c}_{seg}")
                nc.scalar.activation(L_sb[:], a_sb[:], AF.Ln, bias=1.0)
                dl_ps = psH.tile([128, RC], f32, tag="h", name=f"dl{c}_{seg}")
                nc.tensor.matmul(
                    dl_ps[:], cs("dl", 128 * seg, 2, 128), d2_c[:],
                    start=True, stop=True)
                xs = cpool.tile([128, RC], f32, tag="x", name=f"x{c}_{seg}")
                nc.vector.tensor_tensor(xs[:], L_sb[:], dl_ps[:], op=ALU.mult)
                x_sb[seg] = xs

                for beta in range(4):
                    m_sb = cpool.tile([128, RC], f32, tag="m", name=f"m{c}_{seg}_{beta}")
                    nc.scalar.activation(m_sb[:], rgb_ps[beta][:], AF.Exp, scale=-1.0)
                    p_sb = cpool.tile([128, RC], f32, tag="p", name=f"p{c}_{seg}_{beta}")
                    nc.scalar.activation(p_sb[:], m_sb[:], AF.Ln, bias=1.0)
                    r_sb = rgbpool.tile([128, RC], f32, tag="rgbsb",
                                        name=f"rgbsb{c}_{seg}_{beta}")
                    nc.scalar.activation(r_sb[:], p_sb[:], AF.Exp, scale=-1.0)
                    rgb_sb[seg][beta] = r_sb

            for seg in range(2):
                S_ps = psHC.tile([128, RC], f32, tag="hc", name=f"S{c}_{seg}")
                nc.tensor.matmul(S_ps[:], cs("tri", 0, 128, 128), x_sb[seg][:],
                                 start=True, stop=(seg == 0))
                if seg == 1:
                    nc.tensor.matmul(S_ps[:], cs("sum0", 0, 128, 128), x_sb[0][:],
                                     start=False, stop=True)
                tmp = cpool.tile([128, RC], f32, tag="tmp", name=f"tmp{c}_{seg}")
                nc.vector.tensor_tensor(tmp[:], x_sb[seg][:], S_ps[:], op=ALU.subtract)
                E_sb = cpool.tile([128, RC], f32, tag="E", name=f"E{c}_{seg}")
                nc.scalar.activation(E_sb[:], tmp[:], AF.Exp)
                y1_sb = cpool.tile([128, RC], f32, tag="y1", name=f"y1{c}_{seg}")
                nc.scalar.activation(y1_sb[:], x_sb[seg][:], AF.Exp, scale=-1.0)
                t2 = cpool.tile([128, RC], f32, tag="t2", name=f"t2{c}_{seg}")
                nc.vector.tensor_scalar(t2[:], y1_sb[:], -1.0, 1.0,
                                        op0=ALU.mult, op1=ALU.add)
                ws_ = cpool.tile([128, RC], f32, tag="w", name=f"w{c}_{seg}")
                nc.vector.tensor_tensor(ws_[:], t2[:], E_sb[:], op=ALU.mult)
                w_sb[seg] = ws_

            img_ps = psRgb.tile([6, RC], f32, tag="rgb", name=f"img{c}")
            n_img = 0
            for seg in range(2):
                for beta in range(4):
                    wrep_ps = psSig.tile([128, RC], f32, tag="sig",
                                         name=f"wrep{c}_{seg}_{beta}")
                    nc.tensor.matmul(wrep_ps[:], cs("sel", 128 * beta, 128, 128),
                                     w_sb[seg][:], start=True, stop=True)
                    wrgb = cpool.tile([128, RC], f32, tag="wrgb",
                                      name=f"wrgb{c}_{seg}_{beta}")
                    nc.vector.tensor_tensor(wrgb[:], rgb_sb[seg][beta][:],
                                            wrep_ps[:], op=ALU.mult)
                    nc.tensor.matmul(img_ps[:], cs("red", 0, 128, 6), wrgb[:],
                                     start=(n_img == 0), stop=(n_img == 7),
                                     skip_group_check=True)
                    n_img += 1

            wd_ps = psRgb.tile([4, RC], f32, tag="rgb", name=f"wd{c}")
            nc.tensor.matmul(wd_ps[:], cs("wd", 0, 128, 4), w_sb[0][:],
                             start=True, stop=False)
            nc.tensor.matmul(wd_ps[:], cs("wd", 4, 128, 4), w_sb[1][:],
                             start=False, stop=True)

            img_sb = opool.tile([6, RC], f32, tag="img", name=f"imgsb{c}")
            nc.scalar.activation(img_sb[:], img_ps[:], AF.Copy)
            wd_sb = opool.tile([4, RC], f32, tag="wd", name=f"wdsb{c}")
            nc.scalar.activation(wd_sb[:], wd_ps[:], AF.Copy)

            y_ps = psRgb.tile([9, RC], f32, tag="rgb", name=f"y{c}")
            nc.tensor.matmul(y_ps[:], cs("ya", 0, 6, 9), img_sb[:],
                             start=True, stop=False)
            nc.tensor.matmul(y_ps[:], cs("yb", 0, 4, 9), wd_sb[:],
                             start=False, stop=False)
            nc.tensor.matmul(y_ps[:], cs("yb1", 0, 1, 9), cs("one", 0, 1, RC),
                             start=False, stop=True)
            y_sb = opool.tile([9, RC], mybir.dt.float16, tag="ysb", name=f"ysb{c}")
            nc.scalar.activation(y_sb[:], y_ps[:], AF.Copy)
            nc.sync.dma_start(out=y_ap[:, c * RC:(c + 1) * RC], in_=y_sb[:])


_CACHED = {}


def _build_runner(nc):
    """Persistent jitted SPMD runner (avoids bass2jax's per-call re-jit)."""
    import jax
    from jax.sharding import Mesh, PartitionSpec
    from jax.experimental.shard_map import shard_map
    from concourse import bass2jax

    bass2jax.install_neuronx_cc_hook()
    in_names = ["x", "cst"]
    out_names = ["y"]
    out_avals = [jax.core.ShapedArray((9, R_CORE), np.float16)]
    all_names = in_names + out_names
    pname = nc.partition_id_tensor.name if nc.partition_id_tensor else None
    if pname is not None:
        all_names = all_names + [pname]

    def _body(*args):
        operands = list(args)
        if pname is not None:
            operands.append(bass2jax.partition_id_tensor())
        outs = bass2jax._bass_exec_p.bind(
            *operands,
            out_avals=tuple(out_avals),
            in_names=tuple(all_names),
            out_names=tuple(out_names),
            lowering_input_output_aliases=(),
            sim_require_finite=True,
            sim_require_nnan=True,
            nc=nc,
        )
        return tuple(outs)

    import jax.numpy as jnp
    from jax.sharding import NamedSharding

    devices = jax.devices()[:N_CORES]
    mesh = Mesh(np.asarray(devices), ("core",))
    sh = NamedSharding(mesh, PartitionSpec("core"))

    sharded = jax.jit(
        shard_map(_body, mesh=mesh,
                  in_specs=(PartitionSpec("core"),) * 3,
                  out_specs=(PartitionSpec("core"),) * len(out_names),
                  check_rep=False),
        donate_argnums=(2,), keep_unused=True)
    zfn = jax.jit(lambda: jnp.zeros((N_CORES * 9, R_CORE), np.float16),
                  out_shardings=sh)
    zpool = [zfn() for _ in range(4)]
    cst_cache = {}

    def run(x_cat, cst, cst_key):
        if cst_key not in cst_cache:
            cst_cache.clear()
            cst_cache[cst_key] = jax.device_put(
                np.concatenate([cst] * N_CORES, axis=0), sh)
        z = zpool.pop() if zpool else zfn()
        (y_out,) = sharded(x_cat, cst_cache[cst_key], z)
        out = np.asarray(y_out).reshape(N_CORES, 9, R_CORE)
        if len(zpool) < 2:
            zpool.append(zfn())
        return out

    return run


def _build_module():
    import concourse.bacc as bacc
    import concourse.tile as tile
    import concourse.mybir as mybir

    nc = bacc.Bacc("TRN2", target_bir_lowering=False, debug=False)
    x = nc.dram_tensor("x", [12, R_CORE], mybir.dt.float32, kind="ExternalInput")
    cst = nc.dram_tensor("cst", [128, CONST_COLS["total"]], mybir.dt.float32,
                         kind="ExternalInput")
    y = nc.dram_tensor("y", [9, R_CORE], mybir.dt.float16, kind="ExternalOutput")
    with tile.TileContext(nc) as tc:
        emit_nerf(tc, y.ap(), x.ap(), None, cst.ap(), n_rays=R_CORE)
    nc.compile()
    return nc


def kernel(rays_o, rays_d, W1, b1, Wsig, Wsig_d, Wc1, bc1, Wc2, Wc2_d, num_steps):
    import hashlib

    assert int(num_steps) == T
    weights = [np.ascontiguousarray(np.asarray(a, F32))
               for a in (W1, b1, Wsig, Wsig_d, Wc1, bc1, Wc2, Wc2_d)]
    key = hashlib.md5(b"".join(a.tobytes() for a in weights)).hexdigest()

    if _CACHED.get("cst_key") != key:
        C = build_constants(*weights)
        _CACHED["cst"] = pack_const_tile(C)
        _CACHED["cst_key"] = key
    cst = _CACHED["cst"]

    R12 = host_prep(rays_o, rays_d)
    # concat over cores: [N_CORES*12, R_CORE] (shard_map splits on axis 0)
    x_cat = np.ascontiguousarray(
        R12.reshape(12, N_CORES, R_CORE).transpose(1, 0, 2).reshape(
            N_CORES * 12, R_CORE))

    if "run" not in _CACHED:
        _CACHED["nc"] = _build_module()
        _CACHED["run"] = _build_runner(_CACHED["nc"])

    y = _CACHED["run"](x_cat, cst, key)  # [N_CORES, 9, R_CORE] fp16
    out = np.concatenate([y[cidx].T for cidx in range(N_CORES)], axis=0)
    return np.ascontiguousarray(out.astype(np.float32))


if __name__ == "__main__":
    rng = np.random.default_rng(0)
    ins = {
        "rays_o": (rng.random((N_RAYS, 3), dtype=np.float32) - 0.5),
        "rays_d": rng.standard_normal((N_RAYS, 3)).astype(np.float32),
        "W1": rng.standard_normal((3, 32)).astype(np.float32) * 0.5,
        "b1": np.zeros((32,), np.float32),
        "Wsig": rng.standard_normal((32, 1)).astype(np.float32) * 0.5,
        "Wsig_d": rng.standard_normal((32, 1)).astype(np.float32) * 0.5,
        "Wc1": rng.standard_normal((6, 32)).astype(np.float32) * 0.5,
        "bc1": np.zeros((32,), np.float32),
        "Wc2": rng.standard_normal((32, 3)).astype(np.float32) * 0.5,
        "Wc2_d": rng.standard_normal((32, 3)).astype(np.float32) * 0.5,
        "num_steps": 128,
    }
    out = kernel(**ins)
    print("out", out.shape, out.dtype, np.isfinite(out).all())
